# revision 1
# baseline (speedup 1.0000x reference)
"""Mixer (token-mix + channel-mix MLP) kernel for 8 TRN2 NeuronCores.

Strategy (expert-style parallel over the group axes):
  Phase 1 (C-sharded): core m owns channels Cm=[32m,32m+32). LN1 is folded into
  the per-channel PE transpose (augmented matmul whose moving operand is
  [diag(rstd); -mu*rstd] next to an identity, yielding both (x-mu)*rstd and x
  transposed in one matmul); g1/be1 are folded into the fc1 weights/bias on
  the host. Token-mix fc1+fc2 run in bf16 with per-channel [128,128]
  stationary blocks in transposed layout ([feature, batch]); accumulation is
  fp32 in PSUM. The residual u = x + tok is accumulated in fp32 into an SBUF
  staging buffer; LN2 stats come from an fp32 gram matmul on uT with a ones
  column appended ([sum u*u | sum u]).
  AllToAll: the u staging buffer + LN2 stats ship so core k owns patches
  Nk=[32k,32k+32) with all 256 channels.
  Phase 2 (N-sharded): yn = (u-mu2)*rstd2 is materialized (bf16) on the
  receive side from the shipped stats (g2/be2 folded into cw1/bias on host),
  then the channel-mix fc1+fc2 and final fp32 residual run per patch; output
  stays in [patch, channel, batch] layout and the host transposes it back.
"""
import sys
import numpy as np

sys.path.insert(0, "/opt/trn_rl_repo")

import ml_dtypes
import concourse.bass as bass
import concourse.bacc as bacc
import concourse.tile as tile
from concourse import mybir
from concourse.bass_utils import run_bass_kernel_spmd

F32 = mybir.dt.float32
BF16 = mybir.dt.bfloat16
NCORE = 8
B, C, N = 64, 256, 256
CL = C // NCORE   # 32 local channels (phase 1)
NL = N // NCORE   # 32 local patches (phase 2)
EPS = 1e-5
GELU = mybir.ActivationFunctionType.Gelu

HC = CL // 2                     # 16 channels per collective half
A_ELEMS = HC * NL * B            # 32768 u elems per half-block
ST_ELEMS = B * CL * 2            # 4096 stats elems per block
B_ELEMS = A_ELEMS + ST_ELEMS     # second half carries the stats


def build_program(gelu_func=GELU, mmdt=BF16, skip_b2=False, skip_bc2=False):
    nc = bacc.Bacc("TRN2", target_bir_lowering=False, debug=False,
                   enable_asserts=True, num_devices=NCORE)

    x_in = nc.dram_tensor("x_sh", [B, CL, N], F32, kind="ExternalInput")
    wt_in = nc.dram_tensor("wt", [CL, 128, 4, N], mmdt, kind="ExternalInput")
    ct_in = nc.dram_tensor("ct", [NL, 128, 4, C], mmdt, kind="ExternalInput")
    b1t_in = nc.dram_tensor("b1t", [128, 2, CL], F32, kind="ExternalInput")
    b2t_in = nc.dram_tensor("b2t", [128, 2, CL], F32, kind="ExternalInput")
    bc1t_in = nc.dram_tensor("bc1t", [128, 2, NL], F32, kind="ExternalInput")
    bc2t_in = nc.dram_tensor("bc2t", [128, 2, NL], F32, kind="ExternalInput")
    id64_in = nc.dram_tensor("id64", [64, 64], F32, kind="ExternalInput")
    idx_in = nc.dram_tensor("idx65", [65, 64], F32, kind="ExternalInput")

    ybuf = nc.dram_tensor("ybuf", [NL, C, B], F32, kind="ExternalOutput")

    with tile.TileContext(nc) as tc:
        with tc.tile_pool(name="const", bufs=1) as const, \
             tc.tile_pool(name="wpool", bufs=3) as wpool, \
             tc.tile_pool(name="act", bufs=4) as act, \
             tc.tile_pool(name="small", bufs=4) as small, \
             tc.tile_pool(name="dram", bufs=1, space="DRAM") as dram, \
             tc.tile_pool(name="ps", bufs=2, space="PSUM") as ps:

            # two collective halves (by channel) so the first all-to-all
            # overlaps the tail of phase-1 compute; block layout is c-major
            # [c_loc, nl, b]
            send_a = dram.tile([NCORE, A_ELEMS], F32)
            recv_a = dram.tile([NCORE, A_ELEMS], F32)
            send_b = dram.tile([NCORE, B_ELEMS], F32)
            recv_b = dram.tile([NCORE, B_ELEMS], F32)
            halves = [(send_a, recv_a, A_ELEMS), (send_b, recv_b, B_ELEMS)]

            def send_u_view(h, j):
                # [32nl, 16c, 64b]-ordered AP into half h's dest-j block
                st, _, blk = halves[h]
                return bass.AP(tensor=st.tensor, offset=j * blk,
                               ap=[[B, NL], [NL * B, HC], [1, B]])

            def send_st_view(j):
                # [64b, 32c, 2] stats region of send_b's dest-j block
                return bass.AP(tensor=send_b.tensor,
                               offset=j * B_ELEMS + A_ELEMS,
                               ap=[[CL * 2, B], [2, CL], [1, 2]])

            def recv_u_view(h, j):
                # [16c, 32nl, 64b] view of src-core j's u in half h
                _, rt, blk = halves[h]
                return bass.AP(tensor=rt.tensor, offset=j * blk,
                               ap=[[NL * B, HC], [B, NL], [1, B]])

            def recv_st_view(j, comp):
                # [32c, 64b] view of src-core j's stats (layout [b, c, t])
                return bass.AP(tensor=recv_b.tensor,
                               offset=j * B_ELEMS + A_ELEMS + comp,
                               ap=[[2, CL], [CL * 2, B]])

            # ---- constants / persistent tiles ----
            x_aug = const.tile([65, CL, N], F32)      # rows 0-63 = x[b], row 64 = 1
            nc.sync.dma_start(out=x_aug[0:64, :, :], in_=x_in[:])
            nc.vector.memset(x_aug[64:65, :, :], 1.0)
            id64 = const.tile([64, 64], F32)
            nc.sync.dma_start(out=id64[:], in_=id64_in[:])
            idx65 = const.tile([65, 64], F32)
            nc.sync.dma_start(out=idx65[:], in_=idx_in[:])
            b1t = const.tile([128, 2, CL], F32)
            nc.sync.dma_start(out=b1t[:], in_=b1t_in[:])
            b2t = const.tile([128, 2, CL], F32)
            nc.sync.dma_start(out=b2t[:], in_=b2t_in[:])
            bc1t = const.tile([128, 2, NL], F32)
            nc.sync.dma_start(out=bc1t[:], in_=bc1t_in[:])
            bc2t = const.tile([128, 2, NL], F32)
            nc.sync.dma_start(out=bc2t[:], in_=bc2t_in[:])
            eps64 = const.tile([64, 1], F32)
            nc.vector.memset(eps64[:], EPS)

            mv_all = const.tile([64, CL, 2], F32)     # LN1 mean/var
            rstd1_all = const.tile([64, CL], F32)
            nmr1_all = const.tile([64, CL], F32)      # -mu1*rstd1
            mu2_all = const.tile([64, CL], F32)
            var2_all = const.tile([64, CL], F32)
            rstd2_all = const.tile([64, CL], F32)
            nmr2_all = const.tile([64, CL], F32)      # -mu2*rstd2
            # per-channel moving operand: [diag(rstd1); -mu*rstd] | [I64; 0]
            movings2 = const.tile([65, CL, 2, 64], F32)
            u_stage = const.tile([128, 2, CL, 64], F32)   # uT for all channels
            ua = const.tile([128, 2, 65], F32)            # [u | 1] gram rhs
            nc.vector.memset(ua[:, :, 64:65], 1.0)
            sum_all = const.tile([64, CL], F32)           # sum_n u
            esq_all = const.tile([64, CL], F32)           # E[u^2]
            dg = const.tile([64, 64], F32)                # gram*mask scratch

            # ---- phase 1a: LN1 stats, per-channel so matmuls start early ----
            for c in range(CL):
                st6 = small.tile([64, 6], F32)
                nc.vector.bn_stats(out=st6[:], in_=x_aug[0:64, c, :])
                nc.vector.bn_aggr(out=mv_all[:, c, :], in_=st6[:])
                nc.scalar.activation(out=rstd1_all[:, c:c + 1],
                                     in_=mv_all[:, c, 1:2],
                                     func=mybir.ActivationFunctionType.Sqrt,
                                     bias=eps64[:], scale=1.0)
                nc.vector.reciprocal(out=rstd1_all[:, c:c + 1],
                                     in_=rstd1_all[:, c:c + 1])
            nc.vector.tensor_mul(out=nmr1_all[:], in0=mv_all[:, :, 0],
                                 in1=rstd1_all[:])
            nc.vector.tensor_scalar_mul(out=nmr1_all[:], in0=nmr1_all[:],
                                        scalar1=-1.0)
            for c in range(CL):
                nc.gpsimd.tensor_scalar_mul(out=movings2[0:64, c, 0, :],
                                            in0=id64[:],
                                            scalar1=rstd1_all[:, c:c + 1])
                nc.gpsimd.tensor_copy(out=movings2[:, c, 1, :], in_=idx65[:])
                # [64,1] column -> [1,64] row via partition-collapse DMA
                nc.gpsimd.dma_start(out=movings2[64:65, c, 0, :],
                                    in_=nmr1_all[:, c:c + 1])

            # ---- phase 1b: token mixing per channel ----
            for c in range(CL):
                w12 = wpool.tile([128, 4, N], mmdt, tag="w")
                nc.scalar.dma_start(out=w12[:], in_=wt_in[c])

                # zx[:, blk, 0:64] = (x-mu)*rstd transposed; [64:128] = x.T
                zx = ps.tile([128, 2, 128], F32, tag="zx")
                for blk in range(2):
                    nc.tensor.matmul(
                        zx[:, blk, :],
                        x_aug[:, c, blk * 128:(blk + 1) * 128],
                        movings2[:, c, :, :].rearrange("p a b -> p (a b)"),
                        start=True, stop=True)
                z_sb = act.tile([128, 2, 64], mmdt, tag="z")
                nc.vector.tensor_copy(out=z_sb[:], in_=zx[:, :, 0:64])

                hpre = ps.tile([128, 2, 64], F32, tag="hpre")
                for mb in range(2):
                    for nb in range(2):
                        nc.tensor.matmul(
                            hpre[:, mb, :],
                            w12[:, nb, mb * 128:(mb + 1) * 128],
                            z_sb[:, nb, :],
                            start=(nb == 0), stop=(nb == 1))
                hs = act.tile([128, 2, 64], mmdt, tag="h")
                for mb in range(2):
                    nc.scalar.activation(out=hs[:, mb, :], in_=hpre[:, mb, :],
                                         func=gelu_func, bias=b1t[:, mb, c:c + 1])

                tokp = ps.tile([128, 2, 64], F32, tag="tokp")
                for kb in range(2):
                    for mb in range(2):
                        nc.tensor.matmul(
                            tokp[:, kb, :],
                            w12[:, 2 + mb, kb * 128:(kb + 1) * 128],
                            hs[:, mb, :],
                            start=(mb == 0), stop=(mb == 1))
                # u = x.T + tok (+ tb2): DVE may read only one PSUM input/op
                if skip_b2:
                    nc.vector.tensor_copy(out=u_stage[:, :, c, :],
                                          in_=zx[:, :, 64:128])
                else:
                    for kb in range(2):
                        nc.vector.tensor_scalar(
                            out=u_stage[:, kb, c, :], in0=zx[:, kb, 64:128],
                            scalar1=b2t[:, kb, c:c + 1], scalar2=None,
                            op0=mybir.AluOpType.add)
                nc.vector.tensor_add(out=u_stage[:, :, c, :],
                                     in0=u_stage[:, :, c, :], in1=tokp[:])

                # LN2 stats: gram of uT with a ones column ([sum uu | sum u])
                nc.vector.tensor_copy(out=ua[:, :, 0:64], in_=u_stage[:, :, c, :])
                gs = ps.tile([64, 65], F32, tag="gs")
                for blk in range(2):
                    nc.tensor.matmul(gs[:], u_stage[:, blk, c, :], ua[:, blk, :],
                                     start=(blk == 0), stop=(blk == 1))
                nc.vector.tensor_mul(out=dg[:], in0=gs[:, 0:64], in1=id64[:])
                nc.vector.reduce_sum(out=esq_all[:, c:c + 1], in_=dg[:],
                                     axis=mybir.AxisListType.X)
                nc.vector.tensor_copy(out=sum_all[:, c:c + 1], in_=gs[:, 64:65])

            # ---- phase 1c: LN2 rstd batch + ship u and stats ----
            nc.vector.tensor_scalar(
                out=mu2_all[:], in0=sum_all[:], scalar1=1.0 / N, scalar2=None,
                op0=mybir.AluOpType.mult)
            nc.vector.tensor_scalar(
                out=esq_all[:], in0=esq_all[:], scalar1=1.0 / N, scalar2=None,
                op0=mybir.AluOpType.mult)
            nc.vector.tensor_mul(out=var2_all[:], in0=mu2_all[:],
                                 in1=mu2_all[:])
            nc.vector.tensor_sub(out=var2_all[:], in0=esq_all[:],
                                 in1=var2_all[:])
            nc.scalar.activation(out=rstd2_all[:], in_=var2_all[:],
                                 func=mybir.ActivationFunctionType.Sqrt,
                                 bias=eps64[:], scale=1.0)
            nc.vector.reciprocal(out=rstd2_all[:], in_=rstd2_all[:])
            nc.vector.tensor_mul(out=nmr2_all[:], in0=mu2_all[:],
                                 in1=rstd2_all[:])
            nc.vector.tensor_scalar_mul(out=nmr2_all[:], in0=nmr2_all[:],
                                        scalar1=-1.0)
            stats_il = const.tile([64, CL, 2], F32)
            nc.vector.tensor_copy(out=stats_il[:, :, 0], in_=nmr2_all[:])
            nc.vector.tensor_copy(out=stats_il[:, :, 1], in_=rstd2_all[:])
            for h in range(2):
                for blk in range(2):
                    for jr in range(4):
                        j = blk * 4 + jr
                        nc.sync.dma_start(
                            out=send_u_view(h, j),
                            in_=u_stage[jr * 32:(jr + 1) * 32, blk,
                                        h * HC:(h + 1) * HC, :])
            for j in range(NCORE):
                nc.sync.dma_start(out=send_st_view(j), in_=stats_il[:])

            # ---- collectives (half A can start while phase 1 finishes) ----
            nc.gpsimd.collective_compute(
                "AllToAll", mybir.AluOpType.bypass,
                replica_groups=[list(range(NCORE))],
                ins=[send_a.opt()], outs=[recv_a.opt()])
            nc.gpsimd.collective_compute(
                "AllToAll", mybir.AluOpType.bypass,
                replica_groups=[list(range(NCORE))],
                ins=[send_b.opt()], outs=[recv_b.opt()])

            # ---- phase 2a: normalization maps + u staging [c, b] ----
            nm_map = const.tile([128, 2, 64], F32)
            rs_map = const.tile([128, 2, 64], F32)
            recv_stage = const.tile([128, 2, NL, 64], F32)
            for ch in range(2):
                for jr in range(4):
                    j = 4 * ch + jr
                    nc.sync.dma_start(
                        out=nm_map[jr * 32:(jr + 1) * 32, ch, :],
                        in_=recv_st_view(j, 0))
                    nc.sync.dma_start(
                        out=rs_map[jr * 32:(jr + 1) * 32, ch, :],
                        in_=recv_st_view(j, 1))
                    for h in range(2):
                        nc.sync.dma_start(
                            out=recv_stage[jr * 32 + h * HC:
                                           jr * 32 + (h + 1) * HC, ch, :, :],
                            in_=recv_u_view(h, j))
            y_stage = const.tile([128, 2, NL, 64], F32)

            # ---- phase 2b: channel mixing per patch ----
            for nl in range(NL):
                c12 = wpool.tile([128, 4, C], mmdt, tag="w")
                nc.scalar.dma_start(out=c12[:], in_=ct_in[nl])

                u_n = recv_stage[:, :, nl, :]
                t2 = act.tile([128, 2, 64], F32, tag="t2")
                nc.vector.tensor_mul(out=t2[:], in0=u_n, in1=rs_map[:])
                yn = act.tile([128, 2, 64], mmdt, tag="yn")
                nc.vector.tensor_add(out=yn[:], in0=t2[:], in1=nm_map[:])

                h2p = ps.tile([128, 2, 64], F32,
                              tag=("hpre", "zx")[nl % 2])
                for ob in range(2):
                    for cb in range(2):
                        nc.tensor.matmul(
                            h2p[:, ob, :],
                            c12[:, cb, ob * 128:(ob + 1) * 128],
                            yn[:, cb, :],
                            start=(cb == 0), stop=(cb == 1))
                h2s = act.tile([128, 2, 64], mmdt, tag="h")
                for ob in range(2):
                    nc.scalar.activation(out=h2s[:, ob, :], in_=h2p[:, ob, :],
                                         func=gelu_func,
                                         bias=bc1t[:, ob, nl:nl + 1])

                chp = ps.tile([128, 2, 64], F32,
                              tag=("tokp", "gs")[nl % 2])
                for kb in range(2):
                    for ob in range(2):
                        nc.tensor.matmul(
                            chp[:, kb, :],
                            c12[:, 2 + ob, kb * 128:(kb + 1) * 128],
                            h2s[:, ob, :],
                            start=(ob == 0), stop=(ob == 1))
                if skip_bc2:
                    nc.vector.tensor_add(out=y_stage[:, :, nl, :], in0=chp[:],
                                         in1=u_n)
                else:
                    t3 = act.tile([128, 2, 64], F32, tag="t3")
                    for kb in range(2):
                        nc.vector.tensor_scalar(
                            out=t3[:, kb, :], in0=chp[:, kb, :],
                            scalar1=bc2t[:, kb, nl:nl + 1], scalar2=None,
                            op0=mybir.AluOpType.add)
                    nc.vector.tensor_add(out=y_stage[:, :, nl, :], in0=t3[:],
                                         in1=u_n)

            # ---- output: ybuf[nl, c, b] from y_stage[(k_lo), kb, nl, b] ----
            for kb in range(2):
                out_ap = bass.AP(tensor=ybuf,
                                 offset=kb * 128 * B,
                                 ap=[[B, 128], [C * B, NL], [1, B]])
                nc.sync.dma_start(out=out_ap, in_=y_stage[:, kb, :, :])

    nc.finalize()
    return nc


def prep_inputs(x, g1, be1, g2, be2, tw1, tb1, tw2, tb2, cw1, cb1, cw2, cb2,
                mmdt_np=ml_dtypes.bfloat16):
    """Host-side sharding + weight folding. Returns in_maps for the 8 cores."""
    f = np.float32
    bf = mmdt_np
    x = np.asarray(x, f)
    g1, be1, g2, be2 = (np.asarray(a, f) for a in (g1, be1, g2, be2))
    tw1, tb1, tw2, tb2 = (np.asarray(a, f) for a in (tw1, tb1, tw2, tb2))
    cw1, cb1, cw2, cb2 = (np.asarray(a, f) for a in (cw1, cb1, cw2, cb2))

    # token-mix fc1: fold g1 into weights, be1 into bias; lhsT layout [c, n, m]
    w1t = (tw1 * g1[None, None, :]).transpose(0, 2, 1)            # [C, N, N]
    bias1 = tb1 + np.einsum('n,cmn->cm', be1, tw1)                # [C, M]
    w2t = tw2.transpose(0, 2, 1)                                  # [c, m, k]
    t1r = w1t.reshape(C, 2, 128, N)
    t2r = w2t.reshape(C, 2, 128, N)
    wt = np.ascontiguousarray(
        np.stack([t1r[:, 0], t1r[:, 1], t2r[:, 0], t2r[:, 1]],
                 axis=2)).astype(bf)                              # [C, 128, 4, N]

    # channel-mix fc1: fold g2 (per-patch scalar) into cw1, be2 into bias
    c1t = (cw1 * g2[:, None, None]).transpose(0, 2, 1)            # [N, C, C]
    biasc1 = cb1 + be2[:, None] * cw1.sum(axis=2)                 # [N, O]
    c2t = cw2.transpose(0, 2, 1)                                  # [n, o, k]
    c1r = c1t.reshape(N, 2, 128, C)
    c2r = c2t.reshape(N, 2, 128, C)
    ct = np.ascontiguousarray(
        np.stack([c1r[:, 0], c1r[:, 1], c2r[:, 0], c2r[:, 1]],
                 axis=2)).astype(bf)                              # [N, 128, 4, C]

    id64 = np.eye(64, dtype=f)
    idx65 = np.vstack([np.eye(64, dtype=f), np.zeros((1, 64), f)])

    def fold_bias(bm):   # [G, 256] -> [128, 2, G]
        return np.ascontiguousarray(bm.T.reshape(2, 128, -1).transpose(1, 0, 2))

    in_maps = []
    for m in range(NCORE):
        cs = slice(m * CL, (m + 1) * CL)
        ns = slice(m * NL, (m + 1) * NL)
        in_maps.append({
            "x_sh": np.ascontiguousarray(x[:, cs, :]),
            "wt": np.ascontiguousarray(wt[cs]),
            "ct": np.ascontiguousarray(ct[ns]),
            "b1t": fold_bias(bias1[cs]),
            "b2t": fold_bias(tb2[cs]),
            "bc1t": fold_bias(biasc1[ns]),
            "bc2t": fold_bias(cb2[ns]),
            "id64": id64,
            "idx65": idx65,
        })
    return in_maps


def assemble_output(results):
    """results: list of per-core dicts with 'ybuf' [NL, C, B] -> y [B, C, N]."""
    y = np.empty((B, C, N), np.float32)
    for k in range(NCORE):
        y[:, :, k * NL:(k + 1) * NL] = results[k]["ybuf"].transpose(2, 1, 0)
    return y


_PROGRAMS = {}


def get_program(skip_b2, skip_bc2):
    key = (skip_b2, skip_bc2)
    if key not in _PROGRAMS:
        _PROGRAMS[key] = build_program(skip_b2=skip_b2, skip_bc2=skip_bc2)
    return _PROGRAMS[key]


def kernel(**inputs):
    skip_b2 = not np.any(np.asarray(inputs["tb2"]))
    skip_bc2 = not np.any(np.asarray(inputs["cb2"]))
    prog = get_program(skip_b2, skip_bc2)
    in_maps = prep_inputs(**inputs)
    res = run_bass_kernel_spmd(prog, in_maps, list(range(NCORE)))
    return assemble_output(res.results)


if __name__ == "__main__":
    from scipy.special import erf

    rng = np.random.RandomState(0)
    s = 0.02
    inputs = dict(
        x=rng.randn(B, C, N).astype(np.float32),
        g1=np.ones(N, np.float32), be1=np.zeros(N, np.float32),
        g2=np.ones(N, np.float32), be2=np.zeros(N, np.float32),
        tw1=(rng.randn(C, N, N) * s).astype(np.float32),
        tb1=np.zeros((C, N), np.float32),
        tw2=(rng.randn(C, N, N) * s).astype(np.float32),
        tb2=np.zeros((C, N), np.float32),
        cw1=(rng.randn(N, C, C) * s).astype(np.float32),
        cb1=np.zeros((N, C), np.float32),
        cw2=(rng.randn(N, C, C) * s).astype(np.float32),
        cb2=np.zeros((N, C), np.float32),
    )

    def np_ref(x, g1, be1, g2, be2, tw1, tb1, tw2, tb2, cw1, cb1, cw2, cb2):
        def ln(z, g, b):
            mu = z.mean(-1, keepdims=True)
            var = z.var(-1, keepdims=True)
            return (z - mu) / np.sqrt(var + EPS) * g + b
        def gelu(v):
            return v * 0.5 * (1 + erf(v / np.sqrt(2.0)))
        xn = ln(x, g1, be1)
        h = gelu(np.einsum('bcn,cmn->bcm', xn, tw1) + tb1[None])
        tok = np.einsum('bcm,ckm->bck', h, tw2) + tb2[None]
        x = x + tok
        yn = ln(x, g2, be2)
        h2 = gelu(np.einsum('bcn,noc->bon', yn, cw1) + cb1.T[None])
        ch = np.einsum('bon,nko->bkn', h2, cw2) + cb2.T[None]
        return x + ch

    exp = np_ref(**{k: v.astype(np.float64) for k, v in inputs.items()})
    got = kernel(**inputs)
    err = np.abs(got - exp)
    rel = err.max() / np.abs(exp).max()
    print(f"abs err: {err.max():.3e}  rel(absmax): {rel:.3e}")



# revision 11
# speedup vs baseline: 1.2387x; 1.2387x over previous
"""Mixer (token-mix + channel-mix MLP) kernel for 8 TRN2 NeuronCores.

Strategy (expert-style parallel over the group axes), v2 pipeline:
  Phase 1 (C-sharded): core m owns channels Cm=[32m,32m+32). x ships bf16.
  LN1 stats are computed batched per 16-channel half (square+reduce on DVE),
  and folded into the per-channel PE transpose via a moving operand
  [diag(rstd); -mu*rstd | I64; 0] built from a host skeleton + one broadcast
  multiply + one PE transpose of the -mu*rstd row. Token-mix fc1+fc2 run in
  bf16 ([feature, batch] transposed layout, fp32 PSUM). u = x^T + tok is
  written directly in bf16 (c-major [128, c, kb, b]); LN2 stats come from
  one-hot-column stationary matmuls accumulating [sum u^2 | sum u] into PSUM
  across each half.
  Exchange: per half (16 channels), u + stats are staged into per-dest blocks
  [c16, 34, 64b] bf16 (rows 32/33 = -mu2*rstd2, rstd2) and an AllToAll fires
  while the next half computes. Block strides are chosen so the receive side
  restages each half with ONE DMA into [128 (16j+cl), 34, 64].
  Phase 2 (N-sharded): channel-mix weights are host-permuted to the recv
  channel order; yn = u*rstd2 + nmr2 is built with two broadcast DVE ops per
  half; fc1+fc2 per patch; the bf16 output streams out per (nl, half-block)
  DMA overlapped with compute, and the host unpermutes nothing (the output
  DMA scatters rows to natural channel addresses).
"""
import sys
import numpy as np

sys.path.insert(0, "/opt/trn_rl_repo")

import ml_dtypes
import concourse.bass as bass
import concourse.bacc as bacc
import concourse.tile as tile
from concourse import mybir
from concourse.bass_utils import run_bass_kernel_spmd

F32 = mybir.dt.float32
BF16 = mybir.dt.bfloat16
NCORE = 8
B, C, N = 64, 256, 256
CL = C // NCORE   # 32 local channels (phase 1)
NL = N // NCORE   # 32 local patches (phase 2)
EPS = 1e-5
GELU = mybir.ActivationFunctionType.Gelu
RSQRT = mybir.ActivationFunctionType.Rsqrt

HC = CL // 2                  # 16 channels per collective half
NLR = NL + 2                  # 32 u rows + 2 stats rows per block
CSTR = NLR * B                # 2176: c stride inside a dest block
BLK = HC * CSTR               # 34816 elems per dest block (bf16)
STOFF = NL * B                # 2048: stats row offset inside a c line


def build_program(gelu_func=GELU, skip_b2=False, skip_bc2=False):
    nc = bacc.Bacc("TRN2", target_bir_lowering=False, debug=False,
                   enable_asserts=True, num_devices=NCORE)

    x_in = nc.dram_tensor("x_sh", [B, CL, N], BF16, kind="ExternalInput")
    wt_in = nc.dram_tensor("wt", [CL, 128, 4, N], BF16, kind="ExternalInput")
    ct_in = nc.dram_tensor("ct", [NL, 128, 4, C], BF16, kind="ExternalInput")
    b1t_in = nc.dram_tensor("b1t", [128, 2, CL], F32, kind="ExternalInput")
    b2t_in = nc.dram_tensor("b2t", [128, 2, CL], F32, kind="ExternalInput")
    bc1t_in = nc.dram_tensor("bc1t", [128, 2, NL], F32, kind="ExternalInput")
    bc2t_in = nc.dram_tensor("bc2t", [128, 2, NL], F32, kind="ExternalInput")
    skel_in = nc.dram_tensor("skel", [65, CL, 2, 64], BF16, kind="ExternalInput")
    ones_in = nc.dram_tensor("onesel", [128, HC, HC], BF16, kind="ExternalInput")
    id64_in = nc.dram_tensor("id64", [64, 64], F32, kind="ExternalInput")

    ybuf = nc.dram_tensor("ybuf", [NL, C, B], BF16, kind="ExternalOutput")

    with tile.TileContext(nc) as tc:
        with tc.tile_pool(name="const", bufs=1) as const, \
             tc.tile_pool(name="wpool", bufs=3) as wpool, \
             tc.tile_pool(name="act", bufs=4) as act, \
             tc.tile_pool(name="small", bufs=2) as small, \
             tc.tile_pool(name="dram", bufs=1, space="DRAM") as dram, \
             tc.tile_pool(name="ps", bufs=2, space="PSUM") as ps, \
             tc.tile_pool(name="pstat", bufs=1, space="PSUM") as pstat:

            send = [dram.tile([NCORE, BLK], BF16, name=f"send{h}",
                              tag=f"send{h}") for h in range(2)]
            recv = [dram.tile([NCORE, BLK], BF16, name=f"recv{h}",
                              tag=f"recv{h}") for h in range(2)]
            scr = [dram.tile([HC * 2 * B], BF16, name=f"scr{h}",
                             tag=f"scr{h}") for h in range(2)]

            # ---- constants / persistent tiles ----
            x_aug = const.tile([65, CL, N], BF16)   # rows 0-63 = x, row 64 = 1
            nc.sync.dma_start(out=x_aug[0:64, :, :], in_=x_in[:])
            nc.vector.memset(x_aug[64:65, :, :], 1.0)
            # moving operand skeleton: [diag; 0 | I64; 0] per channel
            movings2 = const.tile([65, CL, 2, 64], BF16)
            nc.scalar.dma_start(out=movings2[:], in_=skel_in[:])
            onesel = const.tile([128, HC, HC], BF16)
            nc.sync.dma_start(out=onesel[:], in_=ones_in[:])
            id64 = const.tile([64, 64], F32)
            nc.sync.dma_start(out=id64[:], in_=id64_in[:])
            b1t = const.tile([128, 2, CL], F32)
            nc.sync.dma_start(out=b1t[:], in_=b1t_in[:])
            b2t = const.tile([128, 2, CL], F32)
            nc.sync.dma_start(out=b2t[:], in_=b2t_in[:])
            bc1t = const.tile([128, 2, NL], F32)
            nc.sync.dma_start(out=bc1t[:], in_=bc1t_in[:])
            bc2t = const.tile([128, 2, NL], F32)
            nc.sync.dma_start(out=bc2t[:], in_=bc2t_in[:])
            eps64 = const.tile([64, 1], F32)
            nc.vector.memset(eps64[:], EPS)

            rstd1_all = const.tile([64, CL], F32)
            nmr1_all = const.tile([64, CL], F32)     # -mu1*rstd1
            u_bf = const.tile([128, CL, 2, 64], BF16)  # uT, c-major
            yn_all = const.tile([128, 2, NL, 64], BF16)
            ub = [const.tile([128, NLR, 64], BF16, name=f"ub{h}",
                             tag=f"ub{h}") for h in range(2)]

            rings = [nc.sync, nc.scalar]

            for h in range(2):
                sl = slice(h * HC, (h + 1) * HC)

                # ---- LN1 stats, batched over the half ----
                sq = small.tile([64, HC, N], BF16, tag="sq")
                nc.vector.tensor_mul(out=sq[:], in0=x_aug[0:64, sl, :],
                                     in1=x_aug[0:64, sl, :])
                esq1 = small.tile([64, HC], F32, tag="esq1")
                nc.vector.tensor_reduce(out=esq1[:], in_=sq[:],
                                        axis=mybir.AxisListType.X,
                                        op=mybir.AluOpType.add)
                mu1 = small.tile([64, HC], F32, tag="mu1")
                nc.vector.tensor_reduce(out=mu1[:], in_=x_aug[0:64, sl, :],
                                        axis=mybir.AxisListType.X,
                                        op=mybir.AluOpType.add)
                nc.vector.tensor_scalar(out=mu1[:], in0=mu1[:],
                                        scalar1=1.0 / N, scalar2=None,
                                        op0=mybir.AluOpType.mult)
                nc.vector.tensor_scalar(out=esq1[:], in0=esq1[:],
                                        scalar1=1.0 / N, scalar2=None,
                                        op0=mybir.AluOpType.mult)
                var1 = small.tile([64, HC], F32, tag="var1")
                nc.vector.tensor_mul(out=var1[:], in0=mu1[:], in1=mu1[:])
                nc.vector.tensor_sub(out=var1[:], in0=esq1[:], in1=var1[:])
                nc.scalar.activation(out=rstd1_all[:, sl], in_=var1[:],
                                     func=mybir.ActivationFunctionType.Sqrt,
                                     bias=eps64[:], scale=1.0)
                nc.vector.reciprocal(out=rstd1_all[:, sl],
                                     in_=rstd1_all[:, sl])
                nc.vector.scalar_tensor_tensor(
                    out=nmr1_all[:, sl], in0=mu1[:], scalar=-1.0,
                    in1=rstd1_all[:, sl], op0=mybir.AluOpType.mult,
                    op1=mybir.AluOpType.mult)

                # ---- finalize the moving operands for this half ----
                # diag block: scale identity rows by rstd1 (broadcast over b)
                nc.vector.tensor_mul(
                    out=movings2[0:64, sl, 0, :],
                    in0=movings2[0:64, sl, 0, :],
                    in1=rstd1_all[:, sl].unsqueeze(2).broadcast_to([64, HC, 64]))
                # -mu*rstd row: transpose [64b, 16c] -> [16c, 64b] on the PE
                nmp = pstat.tile([HC, 64], F32, tag="nmp")
                nc.tensor.matmul(nmp[:], nmr1_all[:, sl], id64[:],
                                 start=True, stop=True)
                nms = small.tile([HC, 64], BF16, tag="nms")
                nc.vector.tensor_copy(out=nms[:], in_=nmp[:])
                nc.sync.dma_start(out=movings2[64:65, sl, 0, :], in_=nms[:])

                # ---- token mixing per channel ----
                stat_ps = pstat.tile([HC, 2, 64], F32, tag="st")
                for cl in range(HC):
                    c = h * HC + cl
                    w12 = wpool.tile([128, 4, N], BF16, tag="w")
                    rings[c % 2].dma_start(out=w12[:], in_=wt_in[c])

                    # zx[:, blk, 0:64] = (x-mu)*rstd transposed; [64:128] = x^T
                    zx = ps.tile([128, 2, 128], F32, tag="zx")
                    for blk in range(2):
                        nc.tensor.matmul(
                            zx[:, blk, :],
                            x_aug[:, c, blk * 128:(blk + 1) * 128],
                            movings2[:, c, :, :].rearrange("p a b -> p (a b)"),
                            start=True, stop=True)
                    z_sb = act.tile([128, 2, 64], BF16, tag="z")
                    nc.vector.tensor_copy(out=z_sb[:], in_=zx[:, :, 0:64])

                    hpre = ps.tile([128, 2, 64], F32, tag="hpre")
                    for mb in range(2):
                        for nb in range(2):
                            nc.tensor.matmul(
                                hpre[:, mb, :],
                                w12[:, nb, mb * 128:(mb + 1) * 128],
                                z_sb[:, nb, :],
                                start=(nb == 0), stop=(nb == 1))
                    hs = act.tile([128, 2, 64], BF16, tag="h")
                    for mb in range(2):
                        nc.scalar.activation(out=hs[:, mb, :],
                                             in_=hpre[:, mb, :],
                                             func=gelu_func,
                                             bias=b1t[:, mb, c:c + 1])

                    tokp = ps.tile([128, 2, 64], F32, tag="tokp")
                    for kb in range(2):
                        for mb in range(2):
                            nc.tensor.matmul(
                                tokp[:, kb, :],
                                w12[:, 2 + mb, kb * 128:(kb + 1) * 128],
                                hs[:, mb, :],
                                start=(mb == 0), stop=(mb == 1))

                    # u = x^T (+ tb2) + tok, written bf16 c-major
                    t = act.tile([128, 2, 64], F32, tag="t")
                    if skip_b2:
                        nc.vector.tensor_copy(out=t[:], in_=zx[:, :, 64:128])
                    else:
                        for kb in range(2):
                            nc.vector.tensor_scalar(
                                out=t[:, kb, :], in0=zx[:, kb, 64:128],
                                scalar1=b2t[:, kb, c:c + 1], scalar2=None,
                                op0=mybir.AluOpType.add)
                    nc.vector.tensor_add(out=u_bf[:, c, :, :], in0=t[:],
                                         in1=tokp[:])

                    # LN2 stats: one-hot stationary accumulates [u^2 | u]
                    u2 = act.tile([128, 2, 64], BF16, tag="u2")
                    nc.vector.tensor_mul(out=u2[:], in0=u_bf[:, c, :, :],
                                         in1=u_bf[:, c, :, :])
                    for kb in range(2):
                        nc.tensor.matmul(
                            stat_ps[:, 0, :], onesel[:, cl, :], u2[:, kb, :],
                            start=(cl == 0 and kb == 0),
                            stop=(cl == HC - 1 and kb == 1),
                            skip_group_check=True)
                        nc.tensor.matmul(
                            stat_ps[:, 1, :], onesel[:, cl, :],
                            u_bf[:, c, kb, :],
                            start=(cl == 0 and kb == 0),
                            stop=(cl == HC - 1 and kb == 1),
                            skip_group_check=True)

                # ---- LN2 stats finalize + ship ----
                mu2 = small.tile([HC, 64], F32, tag="mu2")
                nc.vector.tensor_scalar(out=mu2[:], in0=stat_ps[:, 1, :],
                                        scalar1=1.0 / N, scalar2=None,
                                        op0=mybir.AluOpType.mult)
                esq2 = small.tile([HC, 64], F32, tag="esq2")
                nc.vector.tensor_scalar(out=esq2[:], in0=stat_ps[:, 0, :],
                                        scalar1=1.0 / N, scalar2=None,
                                        op0=mybir.AluOpType.mult)
                var2 = small.tile([HC, 64], F32, tag="var2")
                nc.vector.tensor_mul(out=var2[:], in0=mu2[:], in1=mu2[:])
                nc.vector.tensor_sub(out=var2[:], in0=esq2[:], in1=var2[:])
                stats_bf = small.tile([HC, 2, 64], BF16, tag="stbf")
                rstd2 = small.tile([HC, 64], F32, tag="rstd2")
                nc.scalar.activation(out=rstd2[:], in_=var2[:],
                                     func=mybir.ActivationFunctionType.Sqrt,
                                     bias=eps64[0:HC, :], scale=1.0)
                with nc.allow_low_precision(reason="rstd2 ships bf16 anyway"):
                    nc.vector.reciprocal(out=stats_bf[:, 1, :], in_=rstd2[:])
                nc.vector.scalar_tensor_tensor(
                    out=stats_bf[:, 0, :], in0=mu2[:], scalar=-1.0,
                    in1=stats_bf[:, 1, :], op0=mybir.AluOpType.mult,
                    op1=mybir.AluOpType.mult)
                # stats -> dram scratch -> replicate into all 8 dest blocks
                nc.sync.dma_start(
                    out=bass.AP(tensor=scr[h].tensor, offset=0,
                                ap=[[1, HC * 2 * B]]),
                    in_=stats_bf[:])
                nc.scalar.dma_start(
                    out=bass.AP(tensor=send[h].tensor, offset=STOFF,
                                ap=[[BLK, NCORE], [CSTR, HC], [B, 2],
                                    [1, B]]),
                    in_=bass.AP(tensor=scr[h].tensor, offset=0,
                                ap=[[0, NCORE], [1, HC * 2 * B]]))
                # u staging: one DMA per (kb, jr) dest block
                for kb in range(2):
                    for jr in range(4):
                        rings[kb].dma_start(
                            out=bass.AP(tensor=send[h].tensor,
                                        offset=(kb * 4 + jr) * BLK,
                                        ap=[[B, NL], [CSTR, HC], [1, B]]),
                            in_=u_bf[jr * 32:(jr + 1) * 32, sl, kb, :])

                # ---- collective + receive restage (one DMA) ----
                nc.gpsimd.collective_compute(
                    "AllToAll", mybir.AluOpType.bypass,
                    replica_groups=[list(range(NCORE))],
                    ins=[send[h].opt()], outs=[recv[h].opt()])
                nc.gpsimd.dma_start(
                    out=ub[h][:],
                    in_=bass.AP(tensor=recv[h].tensor, offset=0,
                                ap=[[CSTR, 128], [B, NLR], [1, B]]))
                # yn = u * rstd2 + nmr2 (broadcast over nl)
                nc.vector.tensor_mul(
                    out=yn_all[:, h, :, :], in0=ub[h][:, 0:NL, :],
                    in1=ub[h][:, NL + 1:NL + 2, :].broadcast_to([128, NL, 64]))
                nc.vector.tensor_add(
                    out=yn_all[:, h, :, :], in0=yn_all[:, h, :, :],
                    in1=ub[h][:, NL:NL + 1, :].broadcast_to([128, NL, 64]))

            # ---- phase 2: channel mixing per patch ----
            for nl in range(NL):
                c12 = wpool.tile([128, 4, C], BF16, tag="w")
                nc.scalar.dma_start(out=c12[:], in_=ct_in[nl])

                h2p = ps.tile([128, 2, 64], F32, tag=("hpre", "zx")[nl % 2])
                for ob in range(2):
                    for cb in range(2):
                        nc.tensor.matmul(
                            h2p[:, ob, :],
                            c12[:, cb, ob * 128:(ob + 1) * 128],
                            yn_all[:, cb, nl, :],
                            start=(cb == 0), stop=(cb == 1))
                h2s = act.tile([128, 2, 64], BF16, tag="h")
                for ob in range(2):
                    nc.scalar.activation(out=h2s[:, ob, :], in_=h2p[:, ob, :],
                                         func=gelu_func,
                                         bias=bc1t[:, ob, nl:nl + 1])

                if nl % 2 == 0:
                    chp = ps.tile([128, 2, 64], F32, tag="tokp")
                else:
                    chp = pstat.tile([128, 2, 64], F32, tag="st")
                for hb in range(2):
                    for ob in range(2):
                        nc.tensor.matmul(
                            chp[:, hb, :],
                            c12[:, 2 + ob, hb * 128:(hb + 1) * 128],
                            h2s[:, ob, :],
                            start=(ob == 0), stop=(ob == 1))
                y_t = act.tile([128, 2, 64], BF16, tag="yt")
                for hb in range(2):
                    if skip_bc2:
                        nc.vector.tensor_add(out=y_t[:, hb, :],
                                             in0=chp[:, hb, :],
                                             in1=ub[hb][:, nl, :])
                    else:
                        t3 = act.tile([128, 2, 64], F32, tag="t3")
                        nc.vector.tensor_scalar(
                            out=t3[:, hb, :], in0=chp[:, hb, :],
                            scalar1=bc2t[:, hb, nl:nl + 1], scalar2=None,
                            op0=mybir.AluOpType.add)
                        nc.vector.tensor_add(out=y_t[:, hb, :],
                                             in0=t3[:, hb, :],
                                             in1=ub[hb][:, nl, :])
                    # scatter rows to natural channel addresses
                    nc.sync.dma_start(
                        out=bass.AP(tensor=ybuf,
                                    offset=nl * C * B + hb * HC * B,
                                    ap=[[CL * B, NCORE], [B, HC],
                                        [1, B]]),
                        in_=y_t[:, hb, :])

    nc.finalize()
    return nc


def prep_inputs(x, g1, be1, g2, be2, tw1, tb1, tw2, tb2, cw1, cb1, cw2, cb2):
    """Host-side sharding + weight folding. Returns in_maps for the 8 cores."""
    f = np.float32
    bf = ml_dtypes.bfloat16
    x = np.asarray(x, f)
    g1, be1, g2, be2 = (np.asarray(a, f) for a in (g1, be1, g2, be2))
    tw1, tb1, tw2, tb2 = (np.asarray(a, f) for a in (tw1, tb1, tw2, tb2))
    cw1, cb1, cw2, cb2 = (np.asarray(a, f) for a in (cw1, cb1, cw2, cb2))

    # token-mix fc1: fold g1 into weights, be1 into bias; lhsT layout [c, n, m]
    w1t = (tw1 * g1[None, None, :]).transpose(0, 2, 1)            # [C, N, N]
    bias1 = tb1 + np.einsum('n,cmn->cm', be1, tw1)                # [C, M]
    w2t = tw2.transpose(0, 2, 1)                                  # [c, m, k]
    t1r = w1t.reshape(C, 2, 128, N)
    t2r = w2t.reshape(C, 2, 128, N)
    wt = np.ascontiguousarray(
        np.stack([t1r[:, 0], t1r[:, 1], t2r[:, 0], t2r[:, 1]],
                 axis=2)).astype(bf)                              # [C, 128, 4, N]

    # recv channel order: row r of half-block hb = channel 32*(r//16)+16*hb+r%16
    perm = np.array([32 * j + 16 * hb + cl
                     for hb in range(2) for j in range(NCORE)
                     for cl in range(HC)])                        # [256]

    # channel-mix fc1: fold g2 (per-patch scalar) into cw1, be2 into bias
    c1t = (cw1 * g2[:, None, None]).transpose(0, 2, 1)            # [N, C_in, O]
    c1t = c1t[:, perm, :]                                         # permute c_in
    biasc1 = cb1 + be2[:, None] * cw1.sum(axis=2)                 # [N, O]
    c2t = cw2.transpose(0, 2, 1)[:, :, perm]                      # [n, o, k_perm]
    c1r = c1t.reshape(N, 2, 128, C)
    c2r = c2t.reshape(N, 2, 128, C)
    ct = np.ascontiguousarray(
        np.stack([c1r[:, 0], c1r[:, 1], c2r[:, 0], c2r[:, 1]],
                 axis=2)).astype(bf)                              # [N, 128, 4, C]
    bc2p = cb2[:, perm]                                           # [N, K]

    # moving-operand skeleton: identity rows 0-63, zero row 64, both blocks
    skel = np.zeros((65, CL, 2, 64), bf)
    idx = np.arange(64)
    for t in range(2):
        skel[idx, :, t, idx] = 1.0

    onesel = np.zeros((128, HC, HC), bf)
    onesel[:, idx[:HC], idx[:HC]] = 1.0
    id64 = np.eye(64, dtype=f)

    def fold_bias(bm):   # [G, 256] -> [128, 2, G]
        return np.ascontiguousarray(bm.T.reshape(2, 128, -1).transpose(1, 0, 2))

    in_maps = []
    for m in range(NCORE):
        cs = slice(m * CL, (m + 1) * CL)
        ns = slice(m * NL, (m + 1) * NL)
        in_maps.append({
            "x_sh": np.ascontiguousarray(x[:, cs, :]).astype(bf),
            "wt": np.ascontiguousarray(wt[cs]),
            "ct": np.ascontiguousarray(ct[ns]),
            "b1t": fold_bias(bias1[cs]),
            "b2t": fold_bias(tb2[cs]),
            "bc1t": fold_bias(biasc1[ns]),
            "bc2t": fold_bias(bc2p[ns]),
            "skel": skel,
            "onesel": onesel,
            "id64": id64,
        })
    return in_maps


def assemble_output(results):
    """results: list of per-core dicts with 'ybuf' [NL, C, B] -> y [B, C, N]."""
    y = np.empty((B, C, N), np.float32)
    for k in range(NCORE):
        y[:, :, k * NL:(k + 1) * NL] = \
            results[k]["ybuf"].astype(np.float32).transpose(2, 1, 0)
    return y


_PROGRAMS = {}


def get_program(skip_b2, skip_bc2):
    key = (skip_b2, skip_bc2)
    if key not in _PROGRAMS:
        _PROGRAMS[key] = build_program(skip_b2=skip_b2, skip_bc2=skip_bc2)
    return _PROGRAMS[key]


def kernel(**inputs):
    skip_b2 = not np.any(np.asarray(inputs["tb2"]))
    skip_bc2 = not np.any(np.asarray(inputs["cb2"]))
    prog = get_program(skip_b2, skip_bc2)
    in_maps = prep_inputs(**inputs)
    res = run_bass_kernel_spmd(prog, in_maps, list(range(NCORE)))
    return assemble_output(res.results)


if __name__ == "__main__":
    from scipy.special import erf

    rng = np.random.RandomState(0)
    s = 0.02
    inputs = dict(
        x=rng.randn(B, C, N).astype(np.float32),
        g1=np.ones(N, np.float32), be1=np.zeros(N, np.float32),
        g2=np.ones(N, np.float32), be2=np.zeros(N, np.float32),
        tw1=(rng.randn(C, N, N) * s).astype(np.float32),
        tb1=np.zeros((C, N), np.float32),
        tw2=(rng.randn(C, N, N) * s).astype(np.float32),
        tb2=np.zeros((C, N), np.float32),
        cw1=(rng.randn(N, C, C) * s).astype(np.float32),
        cb1=np.zeros((N, C), np.float32),
        cw2=(rng.randn(N, C, C) * s).astype(np.float32),
        cb2=np.zeros((N, C), np.float32),
    )

    def np_ref(x, g1, be1, g2, be2, tw1, tb1, tw2, tb2, cw1, cb1, cw2, cb2):
        def ln(z, g, b):
            mu = z.mean(-1, keepdims=True)
            var = z.var(-1, keepdims=True)
            return (z - mu) / np.sqrt(var + EPS) * g + b
        def gelu(v):
            return v * 0.5 * (1 + erf(v / np.sqrt(2.0)))
        xn = ln(x, g1, be1)
        h = gelu(np.einsum('bcn,cmn->bcm', xn, tw1) + tb1[None])
        tok = np.einsum('bcm,ckm->bck', h, tw2) + tb2[None]
        x = x + tok
        yn = ln(x, g2, be2)
        h2 = gelu(np.einsum('bcn,noc->bon', yn, cw1) + cb1.T[None])
        ch = np.einsum('bon,nko->bkn', h2, cw2) + cb2.T[None]
        return x + ch

    exp = np_ref(**{k: v.astype(np.float64) for k, v in inputs.items()})
    got = kernel(**inputs)
    err = np.abs(got - exp)
    rel = err.max() / np.abs(exp).max()
    print(f"abs err: {err.max():.3e}  rel(absmax): {rel:.3e}")


# revision 23
# speedup vs baseline: 1.7375x; 1.4027x over previous
"""Mixer (token-mix + channel-mix MLP) kernel for 8 TRN2 NeuronCores.

Strategy (expert-style parallel over the group axes), v3 pipeline:
  Phase 1 (C-sharded): core m owns channels Cm=[32m,32m+32). x ships bf16.
  An xT pass transposes each channel on the PE (identity moving operand) and
  accumulates LN1 [sum x^2 | sum x] via one-hot-column stationary matmuls,
  16 channels per PSUM accumulator half. Stats finalize on DVE, ship through
  a DRAM scratch and are replicated to all 128 partitions with a 0-stride
  DMA, so xn = xT*rstd1 + nmr1 is two broadcast DVE ops per channel.
  The main loop is software-pipelined (fc1(c) | fc2(c-1) | LN2-stats(c-2))
  so the PE never waits on the gelu/DVE round trips. u = xT + tok is written
  bf16 c-major; LN2 stats use the same one-hot matmul machinery.
  Exchange: per 16-channel half, u + LN2 stats are staged into per-dest
  blocks [c16, 34, 64b] bf16 (rows 32/33 = -mu2*rstd2, rstd2); the first
  AllToAll fires at mid-loop and overlaps the second half's compute. Block
  strides let the receive side restage each half with ONE DMA into
  [128 (16j+cl), 34, 64].
  Phase 2 (N-sharded): channel-mix weights are host-permuted to the recv
  channel order; yn = u*rstd2 + nmr2 via broadcast DVE ops; fc1/fc2 are
  software-pipelined the same way; bf16 output accumulates in y_stage and
  leaves in 4 large DMAs that scatter rows to natural channel addresses.
"""
import sys
import numpy as np

sys.path.insert(0, "/opt/trn_rl_repo")

import ml_dtypes
import concourse.bass as bass
import concourse.bacc as bacc
import concourse.tile as tile
from concourse import mybir
from concourse.bass_utils import run_bass_kernel_spmd

F32 = mybir.dt.float32
BF16 = mybir.dt.bfloat16
NCORE = 8
B, C, N = 64, 256, 256
CL = C // NCORE   # 32 local channels (phase 1)
NL = N // NCORE   # 32 local patches (phase 2)
EPS = 1e-5
GELU = mybir.ActivationFunctionType.Gelu
SQRT = mybir.ActivationFunctionType.Sqrt
ADD = mybir.AluOpType.add
MUL = mybir.AluOpType.mult

HC = CL // 2                  # 16 channels per collective half
NLR = NL + 2                  # 32 u rows + 2 stats rows per block
CSTR = NLR * B                # 2176: c stride inside a dest block
BLK = HC * CSTR               # 34816 elems per dest block (bf16)
STOFF = NL * B                # 2048: stats row offset inside a c line


def build_program(gelu_func=GELU, skip_b2=False, skip_bc2=False):
    nc = bacc.Bacc("TRN2", target_bir_lowering=False, debug=False,
                   enable_asserts=True, num_devices=NCORE)

    # x packed 128-partition: row p = batch b + 64*(c//16), col cc = c%16
    x_in = nc.dram_tensor("x_sh", [128, HC, N], BF16, kind="ExternalInput")
    wt_in = nc.dram_tensor("wt", [CL, 128, 4, N], BF16, kind="ExternalInput")
    ct_in = nc.dram_tensor("ct", [NL, 128, 4, C], BF16, kind="ExternalInput")
    b1t_in = nc.dram_tensor("b1t", [128, 2, CL], F32, kind="ExternalInput")
    b2t_in = nc.dram_tensor("b2t", [128, 2, CL], F32, kind="ExternalInput")
    bc1t_in = nc.dram_tensor("bc1t", [128, 2, NL], F32, kind="ExternalInput")
    bc2t_in = nc.dram_tensor("bc2t", [128, 2, NL], F32, kind="ExternalInput")
    ones_in = nc.dram_tensor("onesel", [128, HC, HC], BF16, kind="ExternalInput")
    id64_in = nc.dram_tensor("id64", [128, 64], BF16, kind="ExternalInput")

    ybuf = nc.dram_tensor("ybuf", [C, NL, B], BF16, kind="ExternalOutput")
    dbg = False

    with tile.TileContext(nc) as tc:
        with tc.tile_pool(name="const", bufs=1) as const, \
             tc.tile_pool(name="wpool", bufs=4) as wpool, \
             tc.tile_pool(name="act", bufs=4) as act, \
             tc.tile_pool(name="small", bufs=2) as small, \
             tc.tile_pool(name="dram", bufs=1, space="DRAM") as dram, \
             tc.tile_pool(name="ps", bufs=2, space="PSUM") as ps, \
             tc.tile_pool(name="pstat", bufs=1, space="PSUM") as pstat:

            send = [dram.tile([NCORE, BLK], BF16, name=f"send{h}",
                              tag=f"send{h}") for h in range(2)]
            recv = [dram.tile([NCORE, BLK], BF16, name=f"recv{h}",
                              tag=f"recv{h}") for h in range(2)]
            scr1 = [dram.tile([HC * 2 * B], BF16, name=f"scr1{h}",
                              tag=f"scr1{h}") for h in range(2)]
            scr2 = [dram.tile([HC * 2 * B], BF16, name=f"scr2{h}",
                              tag=f"scr2{h}") for h in range(2)]

            # ---- constants / persistent tiles ----
            x_a = const.tile([128, HC, N], BF16)
            nc.sync.dma_start(out=x_a[:], in_=x_in[:])
            onesel = const.tile([128, HC, HC], BF16)
            nc.scalar.dma_start(out=onesel[:], in_=ones_in[:])
            id64 = const.tile([128, 64], BF16)
            nc.scalar.dma_start(out=id64[:], in_=id64_in[:])
            b1t = const.tile([128, 2, CL], F32)
            nc.scalar.dma_start(out=b1t[:], in_=b1t_in[:])
            b2t = const.tile([128, 2, CL], F32)
            nc.scalar.dma_start(out=b2t[:], in_=b2t_in[:])
            bc1t = const.tile([128, 2, NL], F32)
            nc.scalar.dma_start(out=bc1t[:], in_=bc1t_in[:])
            bc2t = const.tile([128, 2, NL], F32)
            nc.scalar.dma_start(out=bc2t[:], in_=bc2t_in[:])
            eps64 = const.tile([64, 1], F32)
            nc.vector.memset(eps64[:], EPS)

            # combined [c, kb, (sq|val), b]: t=0 squares, t=1 values
            xt_all = const.tile([128, CL, 2, 2, 64], BF16)
            u_bf = const.tile([128, CL, 2, 2, 64], BF16)
            rn = [const.tile([128, HC, 2, 64], BF16, name=f"rn{h}",
                             tag=f"rn{h}") for h in range(2)]
            yn_all = const.tile([128, 2, NL, 64], BF16)
            ub = [const.tile([128, NLR, 64], BF16, name=f"ub{h}",
                             tag=f"ub{h}") for h in range(2)]
            y_stage = const.tile([128, 2, NL, 64], BF16)

            st1 = pstat.tile([HC, 2, 64], F32, tag="st1")  # LN1 half A
            st2 = pstat.tile([HC, 2, 64], F32, tag="st2")  # LN1 half B
            stat1 = [st1, st2]

            # ---- xT pass: transpose + LN1 stat accumulation ----
            # one accumulation group per PSUM bank: moving = [x^2 | x]
            def ln1_stats(j):
                stt = stat1[j // HC]
                cl = j % HC
                for kb in range(2):
                    nc.tensor.matmul(
                        stt[:, :, :].rearrange("p a b -> p (a b)"),
                        onesel[:, cl, :],
                        xt_all[:, j, kb, :, :].rearrange("p a b -> p (a b)"),
                        start=(cl == 0 and kb == 0),
                        stop=(cl == HC - 1 and kb == 1),
                        skip_group_check=True)

            for c in range(CL):
                xtp = ps.tile([128, 2, 64], F32, tag="xtp")
                p0 = 64 * (c // HC)
                for blk in range(2):
                    nc.tensor.matmul(
                        xtp[:, blk, :],
                        x_a[p0:p0 + 64, c % HC, blk * 128:(blk + 1) * 128],
                        id64[p0:p0 + 64, :], start=True, stop=True)
                nc.vector.tensor_copy(out=xt_all[:, c, :, 1, :], in_=xtp[:])
                nc.vector.tensor_mul(out=xt_all[:, c, :, 0, :],
                                     in0=xt_all[:, c, :, 1, :],
                                     in1=xt_all[:, c, :, 1, :])
                if c > 1:
                    ln1_stats(c - 2)
            ln1_stats(CL - 2)
            ln1_stats(CL - 1)

            # ---- LN1 finalize per half: rstd/nmr -> replicated rn tiles ----
            def ln_finalize(stt, sbf):
                """sbf[:,0,:] = -mu*rstd (nmr), sbf[:,1,:] = rstd (bf16)."""
                mu = small.tile([HC, 64], F32, tag="mu")
                nc.vector.tensor_scalar(out=mu[:], in0=stt[:, 1, :],
                                        scalar1=1.0 / N, scalar2=None, op0=MUL)
                esq = small.tile([HC, 64], F32, tag="esq")
                nc.vector.tensor_scalar(out=esq[:], in0=stt[:, 0, :],
                                        scalar1=1.0 / N, scalar2=None, op0=MUL)
                var = small.tile([HC, 64], F32, tag="var")
                nc.vector.tensor_mul(out=var[:], in0=mu[:], in1=mu[:])
                nc.vector.tensor_sub(out=var[:], in0=esq[:], in1=var[:])
                rstd = small.tile([HC, 64], F32, tag="rstd")
                nc.scalar.activation(out=rstd[:], in_=var[:], func=SQRT,
                                     bias=eps64[0:HC, :], scale=1.0)
                with nc.allow_low_precision(reason="stats used in bf16"):
                    nc.vector.reciprocal(out=sbf[:, 1, :], in_=rstd[:])
                nc.vector.scalar_tensor_tensor(
                    out=sbf[:, 0, :], in0=mu[:], scalar=-1.0,
                    in1=sbf[:, 1, :], op0=MUL, op1=MUL)

            for h in range(2):
                s1bf = small.tile([HC, 2, 64], BF16, tag="s1bf")
                ln_finalize(stat1[h], s1bf)
                nc.sync.dma_start(
                    out=bass.AP(tensor=scr1[h].tensor, offset=0,
                                ap=[[1, HC * 2 * B]]),
                    in_=s1bf[:])
                # replicate [16c,2t,64b] stats across all 128 partitions
                nc.scalar.dma_start(
                    out=rn[h][:],
                    in_=bass.AP(tensor=scr1[h].tensor, offset=0,
                                ap=[[0, 128], [1, HC * 2 * B]]))

            # ---- main token-mix loop, software-pipelined ----
            # rn layout per partition: [cl, t, b] with t=0 nmr, t=1 rstd
            def emit_xn(c):
                h, cl = c // HC, c % HC
                z = act.tile([128, 2, 64], BF16, tag="z")
                nc.vector.tensor_mul(
                    out=z[:], in0=xt_all[:, c, :, 1, :],
                    in1=rn[h][:, cl, 1:2, :].broadcast_to([128, 2, 64]))
                nc.vector.tensor_add(
                    out=z[:], in0=z[:],
                    in1=rn[h][:, cl, 0:1, :].broadcast_to([128, 2, 64]))
                return z

            w_t, z_t, hs_t = {}, {}, {}

            def emit_w(c):
                w12 = wpool.tile([128, 4, N], BF16, tag="w")
                nc.sync.dma_start(out=w12[:], in_=wt_in[c])
                w_t[c] = w12

            def emit_fc1(c):
                hpre = ps.tile([128, 2, 64], F32, tag="hpre")
                for mb in range(2):
                    for nb in range(2):
                        nc.tensor.matmul(
                            hpre[:, mb, :],
                            w_t[c][:, nb, mb * 128:(mb + 1) * 128],
                            z_t[c][:, nb, :], start=(nb == 0), stop=(nb == 1))
                hs = act.tile([128, 2, 64], BF16, tag="h")
                for mb in range(2):
                    nc.scalar.activation(out=hs[:, mb, :], in_=hpre[:, mb, :],
                                         func=gelu_func,
                                         bias=b1t[:, mb, c:c + 1])
                hs_t[c] = hs

            def emit_fc2(c):
                tokp = ps.tile([128, 2, 64], F32, tag="tokp")
                for kb in range(2):
                    for mb in range(2):
                        nc.tensor.matmul(
                            tokp[:, kb, :],
                            w_t[c][:, 2 + mb, kb * 128:(kb + 1) * 128],
                            hs_t[c][:, mb, :], start=(mb == 0), stop=(mb == 1))
                del w_t[c], hs_t[c]
                if skip_b2:
                    nc.vector.tensor_add(out=u_bf[:, c, :, 1, :],
                                         in0=xt_all[:, c, :, 1, :],
                                         in1=tokp[:])
                else:
                    t = act.tile([128, 2, 64], F32, tag="t")
                    for kb in range(2):
                        nc.vector.tensor_scalar(
                            out=t[:, kb, :], in0=tokp[:, kb, :],
                            scalar1=b2t[:, kb, c:c + 1], scalar2=None, op0=ADD)
                    nc.vector.tensor_add(out=u_bf[:, c, :, 1, :],
                                         in0=xt_all[:, c, :, 1, :], in1=t[:])
                nc.vector.tensor_mul(out=u_bf[:, c, :, 0, :],
                                     in0=u_bf[:, c, :, 1, :],
                                     in1=u_bf[:, c, :, 1, :])

            def emit_ln2(c):
                stt = stat1[c // HC]   # st1/st2 slots reused for LN2
                cl = c % HC
                for kb in range(2):
                    nc.tensor.matmul(
                        stt[:, :, :].rearrange("p a b -> p (a b)"),
                        onesel[:, cl, :],
                        u_bf[:, c, kb, :, :].rearrange("p a b -> p (a b)"),
                        start=(cl == 0 and kb == 0),
                        stop=(cl == HC - 1 and kb == 1),
                        skip_group_check=True)

            def emit_ship(h):
                """LN2 finalize + stage u + stats, then AllToAll + restage."""
                sl = slice(h * HC, (h + 1) * HC)
                s2bf = small.tile([HC, 2, 64], BF16, tag="s2bf")
                ln_finalize(stat1[h], s2bf)
                nc.sync.dma_start(
                    out=bass.AP(tensor=scr2[h].tensor, offset=0,
                                ap=[[1, HC * 2 * B]]),
                    in_=s2bf[:])
                nc.scalar.dma_start(
                    out=bass.AP(tensor=send[h].tensor, offset=STOFF,
                                ap=[[BLK, NCORE], [CSTR, HC], [B, 2], [1, B]]),
                    in_=bass.AP(tensor=scr2[h].tensor, offset=0,
                                ap=[[0, NCORE], [1, HC * 2 * B]]))
                rings = [nc.sync, nc.scalar]
                for kb in range(2):
                    for jr in range(4):
                        rings[kb].dma_start(
                            out=bass.AP(tensor=send[h].tensor,
                                        offset=(kb * 4 + jr) * BLK,
                                        ap=[[B, NL], [CSTR, HC], [1, B]]),
                            in_=u_bf[jr * 32:(jr + 1) * 32, sl, kb, 1, :])
                nc.gpsimd.collective_compute(
                    "AllToAll", mybir.AluOpType.bypass,
                    replica_groups=[list(range(NCORE))],
                    ins=[send[h].opt()], outs=[recv[h].opt()])
                nc.gpsimd.dma_start(
                    out=ub[h][:],
                    in_=bass.AP(tensor=recv[h].tensor, offset=0,
                                ap=[[CSTR, 128], [B, NLR], [1, B]]))

            def emit_yn(h, g0, g1):
                """yn = u*rstd2 + nmr2 for patches [g0, g1)."""
                nc.vector.tensor_mul(
                    out=yn_all[:, h, g0:g1, :], in0=ub[h][:, g0:g1, :],
                    in1=ub[h][:, NL + 1:NL + 2, :].broadcast_to(
                        [128, g1 - g0, 64]))
                nc.vector.tensor_add(
                    out=yn_all[:, h, g0:g1, :], in0=yn_all[:, h, g0:g1, :],
                    in1=ub[h][:, NL:NL + 1, :].broadcast_to(
                        [128, g1 - g0, 64]))

            emit_w(0)
            emit_w(1)
            z_t[0] = emit_xn(0)
            for c in range(CL):
                if c + 2 < CL:
                    emit_w(c + 2)
                if c + 1 < CL:
                    z_t[c + 1] = emit_xn(c + 1)
                emit_fc1(c)
                del z_t[c]
                if c >= 1:
                    emit_fc2(c - 1)
                if c >= 2:
                    emit_ln2(c - 2)
                if c == 17:
                    emit_ship(0)
                if c in (20, 23, 26, 29):
                    g = (c - 20) // 3
                    emit_yn(0, g * 8, (g + 1) * 8)
            emit_fc2(CL - 1)
            emit_ln2(CL - 2)
            emit_ln2(CL - 1)
            emit_ship(1)

            # ---- phase 2: channel mixing per patch, software-pipelined ----
            c_t, h2s_t, chp_t = {}, {}, {}

            def emit_ct(nl):
                c12 = wpool.tile([128, 4, C], BF16, tag="w")
                nc.sync.dma_start(out=c12[:], in_=ct_in[nl])
                c_t[nl] = c12

            def emit_cfc1(nl):
                h2p = ps.tile([128, 2, 64], F32, tag=("hpre", "xtp")[nl % 2])
                for ob in range(2):
                    for cb in range(2):
                        nc.tensor.matmul(
                            h2p[:, ob, :],
                            c_t[nl][:, cb, ob * 128:(ob + 1) * 128],
                            yn_all[:, cb, nl, :],
                            start=(cb == 0), stop=(cb == 1))
                h2s = act.tile([128, 2, 64], BF16, tag="h")
                for ob in range(2):
                    nc.scalar.activation(out=h2s[:, ob, :], in_=h2p[:, ob, :],
                                         func=gelu_func,
                                         bias=bc1t[:, ob, nl:nl + 1])
                h2s_t[nl] = h2s

            def emit_cfc2(nl):
                if nl % 2 == 0:
                    chp = ps.tile([128, 2, 64], F32, tag="tokp")
                else:
                    chp = pstat.tile([128, 2, 64], F32,
                                     tag=("st1", "st2")[(nl // 2) % 2])
                for hb in range(2):
                    for ob in range(2):
                        nc.tensor.matmul(
                            chp[:, hb, :],
                            c_t[nl][:, 2 + ob, hb * 128:(hb + 1) * 128],
                            h2s_t[nl][:, ob, :],
                            start=(ob == 0), stop=(ob == 1))
                del c_t[nl], h2s_t[nl]
                for hb in range(2):
                    if skip_bc2:
                        nc.vector.tensor_add(out=y_stage[:, hb, nl, :],
                                             in0=chp[:, hb, :],
                                             in1=ub[hb][:, nl, :])
                    else:
                        t3 = act.tile([128, 64], F32, tag="t3")
                        nc.vector.tensor_scalar(
                            out=t3[:], in0=chp[:, hb, :],
                            scalar1=bc2t[:, hb, nl:nl + 1], scalar2=None,
                            op0=ADD)
                        nc.vector.tensor_add(out=y_stage[:, hb, nl, :],
                                             in0=t3[:], in1=ub[hb][:, nl, :])

            def emit_out(g):
                # ybuf is [C, NL, B]; row p=16j+cl of half hb -> channel
                # 32j+16hb+cl at address (32j+16hb+cl)*NL*B
                for hb in range(2):
                    nc.scalar.dma_start(
                        out=bass.AP(tensor=ybuf,
                                    offset=hb * HC * NL * B + g * HC * B,
                                    ap=[[CL * NL * B, NCORE],
                                        [NL * B, HC], [1, HC * B]]),
                        in_=y_stage[:, hb, g * HC:(g + 1) * HC, :])

            emit_ct(0)
            emit_ct(1)
            emit_ct(2)
            emit_yn(1, 0, NL)
            for nl in range(NL):
                if nl + 3 < NL:
                    emit_ct(nl + 3)
                emit_cfc1(nl)
                if nl >= 1:
                    emit_cfc2(nl - 1)
                if nl == 17:
                    emit_out(0)
            emit_cfc2(NL - 1)
            emit_out(1)

    nc.finalize()
    return nc


def prep_inputs(x, g1, be1, g2, be2, tw1, tb1, tw2, tb2, cw1, cb1, cw2, cb2):
    """Host-side sharding + weight folding. Returns in_maps for the 8 cores."""
    f = np.float32
    bf = ml_dtypes.bfloat16
    x = np.asarray(x, f)
    g1, be1, g2, be2 = (np.asarray(a, f) for a in (g1, be1, g2, be2))
    tw1, tb1, tw2, tb2 = (np.asarray(a, f) for a in (tw1, tb1, tw2, tb2))
    cw1, cb1, cw2, cb2 = (np.asarray(a, f) for a in (cw1, cb1, cw2, cb2))

    # token-mix fc1: fold g1 into weights, be1 into bias; lhsT layout [c, n, m]
    w1t = (tw1 * g1[None, None, :]).transpose(0, 2, 1)            # [C, N, N]
    bias1 = tb1 + np.einsum('n,cmn->cm', be1, tw1)                # [C, M]
    w2t = tw2.transpose(0, 2, 1)                                  # [c, m, k]
    t1r = w1t.reshape(C, 2, 128, N)
    t2r = w2t.reshape(C, 2, 128, N)
    wt = np.ascontiguousarray(
        np.stack([t1r[:, 0], t1r[:, 1], t2r[:, 0], t2r[:, 1]],
                 axis=2)).astype(bf)                              # [C, 128, 4, N]

    # recv channel order: row r of half-block hb = channel 32*(r//16)+16*hb+r%16
    perm = np.array([32 * j + 16 * hb + cl
                     for hb in range(2) for j in range(NCORE)
                     for cl in range(HC)])                        # [256]

    # channel-mix fc1: fold g2 (per-patch scalar) into cw1, be2 into bias
    c1t = (cw1 * g2[:, None, None]).transpose(0, 2, 1)            # [N, C_in, O]
    c1t = c1t[:, perm, :]                                         # permute c_in
    biasc1 = cb1 + be2[:, None] * cw1.sum(axis=2)                 # [N, O]
    c2t = cw2.transpose(0, 2, 1)[:, :, perm]                      # [n, o, k_perm]
    c1r = c1t.reshape(N, 2, 128, C)
    c2r = c2t.reshape(N, 2, 128, C)
    ct = np.ascontiguousarray(
        np.stack([c1r[:, 0], c1r[:, 1], c2r[:, 0], c2r[:, 1]],
                 axis=2)).astype(bf)                              # [N, 128, 4, C]
    bc2p = cb2[:, perm]                                           # [N, K]

    idx = np.arange(64)
    onesel = np.zeros((128, HC, HC), bf)
    onesel[:, idx[:HC], idx[:HC]] = 1.0
    id64 = np.tile(np.eye(64, dtype=bf), (2, 1))

    def fold_bias(bm):   # [G, 256] -> [128, 2, G]
        return np.ascontiguousarray(bm.T.reshape(2, 128, -1).transpose(1, 0, 2))

    in_maps = []
    for m in range(NCORE):
        cs = slice(m * CL, (m + 1) * CL)
        ns = slice(m * NL, (m + 1) * NL)
        xc = x[:, cs, :]                                  # [B, CL, N]
        xpk = np.concatenate([xc[:, 0:HC, :], xc[:, HC:CL, :]],
                             axis=0)                      # [128, HC, N]
        in_maps.append({
            "x_sh": np.ascontiguousarray(xpk).astype(bf),
            "wt": np.ascontiguousarray(wt[cs]),
            "ct": np.ascontiguousarray(ct[ns]),
            "b1t": fold_bias(bias1[cs]),
            "b2t": fold_bias(tb2[cs]),
            "bc1t": fold_bias(biasc1[ns]),
            "bc2t": fold_bias(bc2p[ns]),
            "onesel": onesel,
            "id64": id64,
        })
    return in_maps


def assemble_output(results):
    """results: list of per-core dicts with 'ybuf' [C, NL, B] -> y [B, C, N]."""
    y = np.empty((B, C, N), np.float32)
    for k in range(NCORE):
        y[:, :, k * NL:(k + 1) * NL] = \
            results[k]["ybuf"].astype(np.float32).transpose(2, 0, 1)
    return y


_PROGRAMS = {}


def get_program(skip_b2, skip_bc2):
    key = (skip_b2, skip_bc2)
    if key not in _PROGRAMS:
        _PROGRAMS[key] = build_program(skip_b2=skip_b2, skip_bc2=skip_bc2)
    return _PROGRAMS[key]


def kernel(**inputs):
    skip_b2 = not np.any(np.asarray(inputs["tb2"]))
    skip_bc2 = not np.any(np.asarray(inputs["cb2"]))
    prog = get_program(skip_b2, skip_bc2)
    in_maps = prep_inputs(**inputs)
    res = run_bass_kernel_spmd(prog, in_maps, list(range(NCORE)))
    return assemble_output(res.results)


if __name__ == "__main__":
    from scipy.special import erf

    rng = np.random.RandomState(0)
    s = 0.02
    inputs = dict(
        x=rng.randn(B, C, N).astype(np.float32),
        g1=np.ones(N, np.float32), be1=np.zeros(N, np.float32),
        g2=np.ones(N, np.float32), be2=np.zeros(N, np.float32),
        tw1=(rng.randn(C, N, N) * s).astype(np.float32),
        tb1=np.zeros((C, N), np.float32),
        tw2=(rng.randn(C, N, N) * s).astype(np.float32),
        tb2=np.zeros((C, N), np.float32),
        cw1=(rng.randn(N, C, C) * s).astype(np.float32),
        cb1=np.zeros((N, C), np.float32),
        cw2=(rng.randn(N, C, C) * s).astype(np.float32),
        cb2=np.zeros((N, C), np.float32),
    )

    def np_ref(x, g1, be1, g2, be2, tw1, tb1, tw2, tb2, cw1, cb1, cw2, cb2):
        def ln(z, g, b):
            mu = z.mean(-1, keepdims=True)
            var = z.var(-1, keepdims=True)
            return (z - mu) / np.sqrt(var + EPS) * g + b
        def gelu(v):
            return v * 0.5 * (1 + erf(v / np.sqrt(2.0)))
        xn = ln(x, g1, be1)
        h = gelu(np.einsum('bcn,cmn->bcm', xn, tw1) + tb1[None])
        tok = np.einsum('bcm,ckm->bck', h, tw2) + tb2[None]
        x = x + tok
        yn = ln(x, g2, be2)
        h2 = gelu(np.einsum('bcn,noc->bon', yn, cw1) + cb1.T[None])
        ch = np.einsum('bon,nko->bkn', h2, cw2) + cb2.T[None]
        return x + ch

    exp = np_ref(**{k: v.astype(np.float64) for k, v in inputs.items()})
    got = kernel(**inputs)
    err = np.abs(got - exp)
    rel = err.max() / np.abs(exp).max()
    print(f"abs err: {err.max():.3e}  rel(absmax): {rel:.3e}")


# revision 32
# speedup vs baseline: 1.7516x; 1.0081x over previous
"""Mixer (token-mix + channel-mix MLP) kernel for 8 TRN2 NeuronCores.

Strategy (expert-style parallel over the group axes), v3 pipeline:
  Phase 1 (C-sharded): core m owns channels Cm=[32m,32m+32). x ships bf16.
  An xT pass transposes each channel on the PE (identity moving operand) and
  accumulates LN1 [sum x^2 | sum x] via one-hot-column stationary matmuls,
  16 channels per PSUM accumulator half. Stats finalize on DVE, ship through
  a DRAM scratch and are replicated to all 128 partitions with a 0-stride
  DMA, so xn = xT*rstd1 + nmr1 is two broadcast DVE ops per channel.
  The main loop is software-pipelined (fc1(c) | fc2(c-1) | LN2-stats(c-2))
  so the PE never waits on the gelu/DVE round trips. u = xT + tok is written
  bf16 c-major; LN2 stats use the same one-hot matmul machinery.
  Exchange: per 16-channel half, u + LN2 stats are staged into per-dest
  blocks [c16, 34, 64b] bf16 (rows 32/33 = -mu2*rstd2, rstd2); the first
  AllToAll fires at mid-loop and overlaps the second half's compute. Block
  strides let the receive side restage each half with ONE DMA into
  [128 (16j+cl), 34, 64].
  Phase 2 (N-sharded): channel-mix weights are host-permuted to the recv
  channel order; yn = u*rstd2 + nmr2 via broadcast DVE ops; fc1/fc2 are
  software-pipelined the same way; bf16 output accumulates in y_stage and
  leaves in 4 large DMAs that scatter rows to natural channel addresses.
"""
import sys
import numpy as np

sys.path.insert(0, "/opt/trn_rl_repo")

import ml_dtypes
import concourse.bass as bass
import concourse.bacc as bacc
import concourse.tile as tile
from concourse import mybir
from concourse.bass_utils import run_bass_kernel_spmd

F32 = mybir.dt.float32
BF16 = mybir.dt.bfloat16
NCORE = 8
B, C, N = 64, 256, 256
CL = C // NCORE   # 32 local channels (phase 1)
NL = N // NCORE   # 32 local patches (phase 2)
EPS = 1e-5
GELU = mybir.ActivationFunctionType.Gelu
SQRT = mybir.ActivationFunctionType.Sqrt
ADD = mybir.AluOpType.add
MUL = mybir.AluOpType.mult

HC = CL // 2                  # 16 channels per collective half
NLR = NL + 2                  # 32 u rows + 2 stats rows per block
CSTR = NLR * B                # 2176: c stride inside a dest block
BLK = HC * CSTR               # 34816 elems per dest block (bf16)
STOFF = NL * B                # 2048: stats row offset inside a c line


def build_program(gelu_func=GELU, skip_b2=False, skip_bc2=False):
    nc = bacc.Bacc("TRN2", target_bir_lowering=False, debug=False,
                   enable_asserts=True, num_devices=NCORE)

    # x packed 128-partition: row p = batch b + 64*(c//16), col cc = c%16
    x_in = nc.dram_tensor("x_sh", [128, HC, N], BF16, kind="ExternalInput")
    wt_in = nc.dram_tensor("wt", [CL, 128, 4, N], BF16, kind="ExternalInput")
    ct_in = nc.dram_tensor("ct", [NL, 128, 4, C], BF16, kind="ExternalInput")
    b1t_in = nc.dram_tensor("b1t", [128, 2, CL], F32, kind="ExternalInput")
    b2t_in = nc.dram_tensor("b2t", [128, 2, CL], F32, kind="ExternalInput")
    bc1t_in = nc.dram_tensor("bc1t", [128, 2, NL], F32, kind="ExternalInput")
    bc2t_in = nc.dram_tensor("bc2t", [128, 2, NL], F32, kind="ExternalInput")
    ones_in = nc.dram_tensor("onesel", [128, HC, HC], BF16, kind="ExternalInput")
    id64_in = nc.dram_tensor("id64", [128, 64], BF16, kind="ExternalInput")

    ybuf = nc.dram_tensor("ybuf", [C, NL, B], BF16, kind="ExternalOutput")
    dbg = False

    with tile.TileContext(nc) as tc:
        with tc.tile_pool(name="const", bufs=1) as const, \
             tc.tile_pool(name="wpool", bufs=4) as wpool, \
             tc.tile_pool(name="act", bufs=6) as act, \
             tc.tile_pool(name="small", bufs=2) as small, \
             tc.tile_pool(name="dram", bufs=1, space="DRAM") as dram, \
             tc.tile_pool(name="ps", bufs=2, space="PSUM") as ps, \
             tc.tile_pool(name="pstat", bufs=1, space="PSUM") as pstat:

            # exchange buffers: one 16-channel group, then two 8-channel
            QBLK = 8 * CSTR
            GSIZE = (BLK, QBLK, QBLK)
            send = [dram.tile([NCORE, GSIZE[i]], BF16, name=f"send{i}",
                              tag=f"send{i}") for i in range(3)]
            recv = [dram.tile([NCORE, GSIZE[i]], BF16, name=f"recv{i}",
                              tag=f"recv{i}") for i in range(3)]
            scr1 = [dram.tile([HC * 2 * B], BF16, name=f"scr1{h}",
                              tag=f"scr1{h}") for h in range(2)]
            scr2 = [dram.tile([HC * 2 * B], BF16, name=f"scr2{i}",
                              tag=f"scr2{i}") for i in range(3)]

            # ---- constants / persistent tiles ----
            x_a = const.tile([128, HC, N], BF16)
            nc.sync.dma_start(out=x_a[:], in_=x_in[:])
            id64 = const.tile([128, 64], BF16)
            nc.scalar.dma_start(out=id64[:], in_=id64_in[:])
            onesel = const.tile([128, HC, HC], BF16)
            nc.scalar.dma_start(out=onesel[:], in_=ones_in[:])
            b1t = const.tile([128, 2, CL], F32)
            nc.scalar.dma_start(out=b1t[:], in_=b1t_in[:])
            b2t = const.tile([128, 2, CL], F32)
            nc.scalar.dma_start(out=b2t[:], in_=b2t_in[:])
            bc1t = const.tile([128, 2, NL], F32)
            nc.scalar.dma_start(out=bc1t[:], in_=bc1t_in[:])
            bc2t = const.tile([128, 2, NL], F32)
            nc.scalar.dma_start(out=bc2t[:], in_=bc2t_in[:])
            eps64 = const.tile([64, 1], F32)
            nc.vector.memset(eps64[:], EPS)

            # combined [c, kb, (sq|val), b]: t=0 squares, t=1 values
            xt_all = const.tile([128, CL, 2, 2, 64], BF16)
            u_bf = const.tile([128, CL, 2, 2, 64], BF16)
            rn = [const.tile([128, HC, 2, 64], BF16, name=f"rn{h}",
                             tag=f"rn{h}") for h in range(2)]
            yn_all = const.tile([128, 2, NL, 64], BF16)
            ub = [const.tile([128, NLR, 64], BF16, name=f"ub{h}",
                             tag=f"ub{h}") for h in range(2)]
            y_stage = const.tile([128, 2, NL, 64], BF16)

            st1 = pstat.tile([HC, 2, 64], F32, tag="st1")  # LN1 half A
            st2 = pstat.tile([HC, 2, 64], F32, tag="st2")  # LN1 half B
            stat1 = [st1, st2]

            # ---- xT pass: transpose + LN1 stat accumulation ----
            # one accumulation group per PSUM bank: moving = [x^2 | x]
            def ln1_stats(j):
                stt = stat1[j // HC]
                cl = j % HC
                for kb in range(2):
                    nc.tensor.matmul(
                        stt[:, :, :].rearrange("p a b -> p (a b)"),
                        onesel[:, cl, :],
                        xt_all[:, j, kb, :, :].rearrange("p a b -> p (a b)"),
                        start=(cl == 0 and kb == 0),
                        stop=(cl == HC - 1 and kb == 1),
                        skip_group_check=True)

            for c in range(CL):
                xtp = ps.tile([128, 2, 64], F32, tag="xtp")
                p0 = 64 * (c // HC)
                for blk in range(2):
                    nc.tensor.matmul(
                        xtp[:, blk, :],
                        x_a[p0:p0 + 64, c % HC, blk * 128:(blk + 1) * 128],
                        id64[p0:p0 + 64, :], start=True, stop=True)
                nc.vector.tensor_copy(out=xt_all[:, c, :, 1, :], in_=xtp[:])
                nc.vector.tensor_mul(out=xt_all[:, c, :, 0, :],
                                     in0=xt_all[:, c, :, 1, :],
                                     in1=xt_all[:, c, :, 1, :])
                if c > 1:
                    ln1_stats(c - 2)
            ln1_stats(CL - 2)
            ln1_stats(CL - 1)

            # ---- LN1 finalize per half: rstd/nmr -> replicated rn tiles ----
            def ln_finalize(stt, sbf):
                """sbf[:,0,:] = -mu*rstd (nmr), sbf[:,1,:] = rstd (bf16)."""
                mu = small.tile([HC, 64], F32, tag="mu")
                nc.vector.tensor_scalar(out=mu[:], in0=stt[:, 1, :],
                                        scalar1=1.0 / N, scalar2=None, op0=MUL)
                esq = small.tile([HC, 64], F32, tag="esq")
                nc.vector.tensor_scalar(out=esq[:], in0=stt[:, 0, :],
                                        scalar1=1.0 / N, scalar2=None, op0=MUL)
                var = small.tile([HC, 64], F32, tag="var")
                nc.vector.tensor_mul(out=var[:], in0=mu[:], in1=mu[:])
                nc.vector.tensor_sub(out=var[:], in0=esq[:], in1=var[:])
                rstd = small.tile([HC, 64], F32, tag="rstd")
                nc.scalar.activation(out=rstd[:], in_=var[:], func=SQRT,
                                     bias=eps64[0:HC, :], scale=1.0)
                with nc.allow_low_precision(reason="stats used in bf16"):
                    nc.vector.reciprocal(out=sbf[:, 1, :], in_=rstd[:])
                nc.vector.scalar_tensor_tensor(
                    out=sbf[:, 0, :], in0=mu[:], scalar=-1.0,
                    in1=sbf[:, 1, :], op0=MUL, op1=MUL)

            for h in range(2):
                s1bf = small.tile([HC, 2, 64], BF16, tag="s1bf")
                ln_finalize(stat1[h], s1bf)
                nc.gpsimd.dma_start(
                    out=bass.AP(tensor=scr1[h].tensor, offset=0,
                                ap=[[1, HC * 2 * B]]),
                    in_=s1bf[:])
                # replicate [16c,2t,64b] stats across all 128 partitions
                nc.gpsimd.dma_start(
                    out=rn[h][:],
                    in_=bass.AP(tensor=scr1[h].tensor, offset=0,
                                ap=[[0, 128], [1, HC * 2 * B]]))

            # ---- main token-mix loop, software-pipelined ----
            # rn layout per partition: [cl, t, b] with t=0 nmr, t=1 rstd
            def emit_xn(c):
                h, cl = c // HC, c % HC
                z = act.tile([128, 2, 64], BF16, tag="z")
                nc.vector.tensor_mul(
                    out=z[:], in0=xt_all[:, c, :, 1, :],
                    in1=rn[h][:, cl, 1:2, :].broadcast_to([128, 2, 64]))
                nc.vector.tensor_add(
                    out=z[:], in0=z[:],
                    in1=rn[h][:, cl, 0:1, :].broadcast_to([128, 2, 64]))
                return z

            w_t, z_t, hs_t = {}, {}, {}

            def emit_w(c):
                w12 = wpool.tile([128, 4, N], BF16, tag="w")
                nc.sync.dma_start(out=w12[:], in_=wt_in[c])
                w_t[c] = w12

            def emit_fc1(c):
                hpre = ps.tile([128, 2, 64], F32, tag="hpre")
                for mb in range(2):
                    for nb in range(2):
                        nc.tensor.matmul(
                            hpre[:, mb, :],
                            w_t[c][:, nb, mb * 128:(mb + 1) * 128],
                            z_t[c][:, nb, :], start=(nb == 0), stop=(nb == 1))
                hs = act.tile([128, 2, 64], BF16, tag="h")
                for mb in range(2):
                    nc.scalar.activation(out=hs[:, mb, :], in_=hpre[:, mb, :],
                                         func=gelu_func,
                                         bias=b1t[:, mb, c:c + 1])
                hs_t[c] = hs

            def emit_fc2(c):
                tokp = ps.tile([128, 2, 64], F32, tag="tokp")
                for kb in range(2):
                    for mb in range(2):
                        nc.tensor.matmul(
                            tokp[:, kb, :],
                            w_t[c][:, 2 + mb, kb * 128:(kb + 1) * 128],
                            hs_t[c][:, mb, :], start=(mb == 0), stop=(mb == 1))
                del w_t[c], hs_t[c]
                if skip_b2:
                    nc.vector.tensor_add(out=u_bf[:, c, :, 1, :],
                                         in0=xt_all[:, c, :, 1, :],
                                         in1=tokp[:])
                else:
                    t = act.tile([128, 2, 64], F32, tag="t")
                    for kb in range(2):
                        nc.vector.tensor_scalar(
                            out=t[:, kb, :], in0=tokp[:, kb, :],
                            scalar1=b2t[:, kb, c:c + 1], scalar2=None, op0=ADD)
                    nc.vector.tensor_add(out=u_bf[:, c, :, 1, :],
                                         in0=xt_all[:, c, :, 1, :], in1=t[:])
                nc.vector.tensor_mul(out=u_bf[:, c, :, 0, :],
                                     in0=u_bf[:, c, :, 1, :],
                                     in1=u_bf[:, c, :, 1, :])

            def emit_ln2(c):
                stt = stat1[c // HC]   # st1/st2 slots reused for LN2
                cl = c % HC
                for kb in range(2):
                    nc.tensor.matmul(
                        stt[:, :, :].rearrange("p a b -> p (a b)"),
                        onesel[:, cl, :],
                        u_bf[:, c, kb, :, :].rearrange("p a b -> p (a b)"),
                        start=(cl == 0 and kb == 0),
                        stop=(cl == HC - 1 and kb == 1),
                        skip_group_check=True)

            def emit_ship(h):
                """LN2 finalize + stage u + stats, then AllToAll + restage."""
                sl = slice(h * HC, (h + 1) * HC)
                s2bf = small.tile([HC, 2, 64], BF16, tag="s2bf")
                ln_finalize(stat1[h], s2bf)
                nc.sync.dma_start(
                    out=bass.AP(tensor=scr2[h].tensor, offset=0,
                                ap=[[1, HC * 2 * B]]),
                    in_=s2bf[:])
                nc.scalar.dma_start(
                    out=bass.AP(tensor=send[h].tensor, offset=STOFF,
                                ap=[[BLK, NCORE], [CSTR, HC], [B, 2], [1, B]]),
                    in_=bass.AP(tensor=scr2[h].tensor, offset=0,
                                ap=[[0, NCORE], [1, HC * 2 * B]]))
                rings = [nc.sync, nc.scalar]
                for kb in range(2):
                    for jr in range(4):
                        rings[kb].dma_start(
                            out=bass.AP(tensor=send[h].tensor,
                                        offset=(kb * 4 + jr) * BLK,
                                        ap=[[B, NL], [CSTR, HC], [1, B]]),
                            in_=u_bf[jr * 32:(jr + 1) * 32, sl, kb, 1, :])
                nc.gpsimd.collective_compute(
                    "AllToAll", mybir.AluOpType.bypass,
                    replica_groups=[list(range(NCORE))],
                    ins=[send[h].opt()], outs=[recv[h].opt()])
                nc.gpsimd.dma_start(
                    out=ub[h][:],
                    in_=bass.AP(tensor=recv[h].tensor, offset=0,
                                ap=[[CSTR, 128], [B, NLR], [1, B]]))

            def emit_yn(gi):
                """yn = u*rstd2 + nmr2 for group gi's 64 ub rows."""
                h, p0 = gi // 2, 64 * (gi % 2)
                pe = p0 + 64
                nc.vector.tensor_mul(
                    out=yn_all[p0:pe, h, :, :], in0=ub[h][p0:pe, 0:NL, :],
                    in1=ub[h][p0:pe, NL + 1:NL + 2, :].broadcast_to(
                        [64, NL, 64]))
                nc.vector.tensor_add(
                    out=yn_all[p0:pe, h, :, :], in0=yn_all[p0:pe, h, :, :],
                    in1=ub[h][p0:pe, NL:NL + 1, :].broadcast_to(
                        [64, NL, 64]))

            for i in range(5):
                emit_w(i)
            z_t[0] = emit_xn(0)
            for c in range(CL):
                if c + 5 < CL:
                    emit_w(c + 5)
                if c + 1 < CL:
                    z_t[c + 1] = emit_xn(c + 1)
                emit_fc1(c)
                del z_t[c]
                if c == 2:
                    ln1_finalize(1)   # rn_b needed from xn(16) only
                if c >= 1:
                    emit_fc2(c - 1)
                if c >= 2:
                    emit_ln2(c - 2)
                if c == 10:
                    emit_fin0(0)
                if c == 17:
                    emit_ship(0)
                if c in (20, 23, 26, 29):
                    g = (c - 20) // 3
                    emit_yn(0, g * 8, (g + 1) * 8)
            emit_fc2(CL - 1)
            emit_ln2(CL - 2)
            emit_ln2(CL - 1)
            emit_ship(1)

            # ---- phase 2: channel mixing per patch, software-pipelined ----
            c_t, h2s_t, chp_t = {}, {}, {}

            def emit_ct(nl):
                c12 = wpool.tile([128, 4, C], BF16, tag="w")
                (nc.sync, nc.scalar)[nl % 2].dma_start(out=c12[:],
                                                       in_=ct_in[nl])
                c_t[nl] = c12

            def emit_cfc1(nl):
                h2p = ps.tile([128, 2, 64], F32, tag=("hpre", "xtp")[nl % 2])
                for ob in range(2):
                    for cb in range(2):
                        nc.tensor.matmul(
                            h2p[:, ob, :],
                            c_t[nl][:, cb, ob * 128:(ob + 1) * 128],
                            yn_all[:, cb, nl, :],
                            start=(cb == 0), stop=(cb == 1))
                h2s = act.tile([128, 2, 64], BF16, tag="h")
                for ob in range(2):
                    nc.scalar.activation(out=h2s[:, ob, :], in_=h2p[:, ob, :],
                                         func=gelu_func,
                                         bias=bc1t[:, ob, nl:nl + 1])
                h2s_t[nl] = h2s

            def emit_cfc2(nl):
                if nl % 2 == 0:
                    chp = ps.tile([128, 2, 64], F32, tag="tokp")
                else:
                    chp = pstat.tile([128, 2, 64], F32,
                                     tag=("st1", "st2")[(nl // 2) % 2])
                for hb in range(2):
                    for ob in range(2):
                        nc.tensor.matmul(
                            chp[:, hb, :],
                            c_t[nl][:, 2 + ob, hb * 128:(hb + 1) * 128],
                            h2s_t[nl][:, ob, :],
                            start=(ob == 0), stop=(ob == 1))
                del c_t[nl], h2s_t[nl]
                for hb in range(2):
                    if skip_bc2:
                        nc.vector.tensor_add(out=y_stage[:, hb, nl, :],
                                             in0=chp[:, hb, :],
                                             in1=ub[hb][:, nl, :])
                    else:
                        t3 = act.tile([128, 64], F32, tag="t3")
                        nc.vector.tensor_scalar(
                            out=t3[:], in0=chp[:, hb, :],
                            scalar1=bc2t[:, hb, nl:nl + 1], scalar2=None,
                            op0=ADD)
                        nc.vector.tensor_add(out=y_stage[:, hb, nl, :],
                                             in0=t3[:], in1=ub[hb][:, nl, :])

            def emit_out(g):
                # ybuf is [C, NL, B]; row p=16j+cl of half hb -> channel
                # 32j+16hb+cl at address (32j+16hb+cl)*NL*B
                for hb in range(2):
                    nc.scalar.dma_start(
                        out=bass.AP(tensor=ybuf,
                                    offset=hb * HC * NL * B + g * HC * B,
                                    ap=[[CL * NL * B, NCORE],
                                        [NL * B, HC], [1, HC * B]]),
                        in_=y_stage[:, hb, g * HC:(g + 1) * HC, :])

            emit_ct(0)
            emit_ct(1)
            emit_ct(2)
            emit_yn(1, 0, NL)
            for nl in range(NL):
                if nl + 3 < NL:
                    emit_ct(nl + 3)
                emit_cfc1(nl)
                if nl >= 1:
                    emit_cfc2(nl - 1)
                if nl == 17:
                    emit_out(0)
            emit_cfc2(NL - 1)
            emit_out(1)

    nc.finalize()
    return nc


def prep_inputs(x, g1, be1, g2, be2, tw1, tb1, tw2, tb2, cw1, cb1, cw2, cb2):
    """Host-side sharding + weight folding. Returns in_maps for the 8 cores."""
    f = np.float32
    bf = ml_dtypes.bfloat16
    x = np.asarray(x, f)
    g1, be1, g2, be2 = (np.asarray(a, f) for a in (g1, be1, g2, be2))
    tw1, tb1, tw2, tb2 = (np.asarray(a, f) for a in (tw1, tb1, tw2, tb2))
    cw1, cb1, cw2, cb2 = (np.asarray(a, f) for a in (cw1, cb1, cw2, cb2))

    # token-mix fc1: fold g1 into weights, be1 into bias; lhsT layout [c, n, m]
    w1t = (tw1 * g1[None, None, :]).transpose(0, 2, 1)            # [C, N, N]
    bias1 = tb1 + np.einsum('n,cmn->cm', be1, tw1)                # [C, M]
    w2t = tw2.transpose(0, 2, 1)                                  # [c, m, k]
    t1r = w1t.reshape(C, 2, 128, N)
    t2r = w2t.reshape(C, 2, 128, N)
    wt = np.ascontiguousarray(
        np.stack([t1r[:, 0], t1r[:, 1], t2r[:, 0], t2r[:, 1]],
                 axis=2)).astype(bf)                              # [C, 128, 4, N]

    # recv channel order: row (hb, q, j, cl) -> channel 32j+16hb+8q+cl
    perm = np.array([32 * j + 16 * hb + 8 * q + cl
                     for hb in range(2) for q in range(2)
                     for j in range(NCORE) for cl in range(8)])    # [256]

    # channel-mix fc1: fold g2 (per-patch scalar) into cw1, be2 into bias
    c1t = (cw1 * g2[:, None, None]).transpose(0, 2, 1)            # [N, C_in, O]
    c1t = c1t[:, perm, :]                                         # permute c_in
    biasc1 = cb1 + be2[:, None] * cw1.sum(axis=2)                 # [N, O]
    c2t = cw2.transpose(0, 2, 1)[:, :, perm]                      # [n, o, k_perm]
    c1r = c1t.reshape(N, 2, 128, C)
    c2r = c2t.reshape(N, 2, 128, C)
    ct = np.ascontiguousarray(
        np.stack([c1r[:, 0], c1r[:, 1], c2r[:, 0], c2r[:, 1]],
                 axis=2)).astype(bf)                              # [N, 128, 4, C]
    bc2p = cb2[:, perm]                                           # [N, K]

    idx = np.arange(64)
    onesel = np.zeros((128, HC, HC), bf)
    onesel[:, idx[:HC], idx[:HC]] = 1.0
    id64 = np.tile(np.eye(64, dtype=bf), (2, 1))

    def fold_bias(bm):   # [G, 256] -> [128, 2, G]
        return np.ascontiguousarray(bm.T.reshape(2, 128, -1).transpose(1, 0, 2))

    in_maps = []
    for m in range(NCORE):
        cs = slice(m * CL, (m + 1) * CL)
        ns = slice(m * NL, (m + 1) * NL)
        xc = x[:, cs, :]                                  # [B, CL, N]
        xpk = np.concatenate([xc[:, 0:HC, :], xc[:, HC:CL, :]],
                             axis=0)                      # [128, HC, N]
        in_maps.append({
            "x_sh": np.ascontiguousarray(xpk).astype(bf),
            "wt": np.ascontiguousarray(wt[cs]),
            "ct": np.ascontiguousarray(ct[ns]),
            "b1t": fold_bias(bias1[cs]),
            "b2t": fold_bias(tb2[cs]),
            "bc1t": fold_bias(biasc1[ns]),
            "bc2t": fold_bias(bc2p[ns]),
            "onesel": onesel,
            "id64": id64,
        })
    return in_maps


def assemble_output(results):
    """results: list of per-core dicts with 'ybuf' [C, NL, B] -> y [B, C, N]."""
    y = np.empty((B, C, N), np.float32)
    for k in range(NCORE):
        y[:, :, k * NL:(k + 1) * NL] = \
            results[k]["ybuf"].astype(np.float32).transpose(2, 0, 1)
    return y


_PROGRAMS = {}


def get_program(skip_b2, skip_bc2):
    key = (skip_b2, skip_bc2)
    if key not in _PROGRAMS:
        _PROGRAMS[key] = build_program(skip_b2=skip_b2, skip_bc2=skip_bc2)
    return _PROGRAMS[key]


def kernel(**inputs):
    skip_b2 = not np.any(np.asarray(inputs["tb2"]))
    skip_bc2 = not np.any(np.asarray(inputs["cb2"]))
    prog = get_program(skip_b2, skip_bc2)
    in_maps = prep_inputs(**inputs)
    res = run_bass_kernel_spmd(prog, in_maps, list(range(NCORE)))
    return assemble_output(res.results)


if __name__ == "__main__":
    from scipy.special import erf

    rng = np.random.RandomState(0)
    s = 0.02
    inputs = dict(
        x=rng.randn(B, C, N).astype(np.float32),
        g1=np.ones(N, np.float32), be1=np.zeros(N, np.float32),
        g2=np.ones(N, np.float32), be2=np.zeros(N, np.float32),
        tw1=(rng.randn(C, N, N) * s).astype(np.float32),
        tb1=np.zeros((C, N), np.float32),
        tw2=(rng.randn(C, N, N) * s).astype(np.float32),
        tb2=np.zeros((C, N), np.float32),
        cw1=(rng.randn(N, C, C) * s).astype(np.float32),
        cb1=np.zeros((N, C), np.float32),
        cw2=(rng.randn(N, C, C) * s).astype(np.float32),
        cb2=np.zeros((N, C), np.float32),
    )

    def np_ref(x, g1, be1, g2, be2, tw1, tb1, tw2, tb2, cw1, cb1, cw2, cb2):
        def ln(z, g, b):
            mu = z.mean(-1, keepdims=True)
            var = z.var(-1, keepdims=True)
            return (z - mu) / np.sqrt(var + EPS) * g + b
        def gelu(v):
            return v * 0.5 * (1 + erf(v / np.sqrt(2.0)))
        xn = ln(x, g1, be1)
        h = gelu(np.einsum('bcn,cmn->bcm', xn, tw1) + tb1[None])
        tok = np.einsum('bcm,ckm->bck', h, tw2) + tb2[None]
        x = x + tok
        yn = ln(x, g2, be2)
        h2 = gelu(np.einsum('bcn,noc->bon', yn, cw1) + cb1.T[None])
        ch = np.einsum('bon,nko->bkn', h2, cw2) + cb2.T[None]
        return x + ch

    exp = np_ref(**{k: v.astype(np.float64) for k, v in inputs.items()})
    got = kernel(**inputs)
    err = np.abs(got - exp)
    rel = err.max() / np.abs(exp).max()
    print(f"abs err: {err.max():.3e}  rel(absmax): {rel:.3e}")


# revision 33
# speedup vs baseline: 1.9463x; 1.1111x over previous
"""Mixer (token-mix + channel-mix MLP) kernel for 8 TRN2 NeuronCores.

Strategy (expert-style parallel over the group axes), v3 pipeline:
  Phase 1 (C-sharded): core m owns channels Cm=[32m,32m+32). x ships bf16.
  An xT pass transposes each channel on the PE (identity moving operand) and
  accumulates LN1 [sum x^2 | sum x] via one-hot-column stationary matmuls,
  16 channels per PSUM accumulator half. Stats finalize on DVE, ship through
  a DRAM scratch and are replicated to all 128 partitions with a 0-stride
  DMA, so xn = xT*rstd1 + nmr1 is two broadcast DVE ops per channel.
  The main loop is software-pipelined (fc1(c) | fc2(c-1) | LN2-stats(c-2))
  so the PE never waits on the gelu/DVE round trips. u = xT + tok is written
  bf16 c-major; LN2 stats use the same one-hot matmul machinery.
  Exchange: per 16-channel half, u + LN2 stats are staged into per-dest
  blocks [c16, 34, 64b] bf16 (rows 32/33 = -mu2*rstd2, rstd2); the first
  AllToAll fires at mid-loop and overlaps the second half's compute. Block
  strides let the receive side restage each half with ONE DMA into
  [128 (16j+cl), 34, 64].
  Phase 2 (N-sharded): channel-mix weights are host-permuted to the recv
  channel order; yn = u*rstd2 + nmr2 via broadcast DVE ops; fc1/fc2 are
  software-pipelined the same way; bf16 output accumulates in y_stage and
  leaves in 4 large DMAs that scatter rows to natural channel addresses.
"""
import sys
import numpy as np

sys.path.insert(0, "/opt/trn_rl_repo")

import ml_dtypes
import concourse.bass as bass
import concourse.bacc as bacc
import concourse.tile as tile
from concourse import mybir
from concourse.bass_utils import run_bass_kernel_spmd

F32 = mybir.dt.float32
BF16 = mybir.dt.bfloat16
NCORE = 8
B, C, N = 64, 256, 256
CL = C // NCORE   # 32 local channels (phase 1)
NL = N // NCORE   # 32 local patches (phase 2)
EPS = 1e-5
GELU = mybir.ActivationFunctionType.Gelu
SQRT = mybir.ActivationFunctionType.Sqrt
ADD = mybir.AluOpType.add
MUL = mybir.AluOpType.mult

HC = CL // 2                  # 16 channels per collective half
NLR = NL + 2                  # 32 u rows + 2 stats rows per block
CSTR = NLR * B                # 2176: c stride inside a dest block
BLK = HC * CSTR               # 34816 elems per dest block (bf16)
STOFF = NL * B                # 2048: stats row offset inside a c line


def build_program(gelu_func=GELU, skip_b2=False, skip_bc2=False):
    nc = bacc.Bacc("TRN2", target_bir_lowering=False, debug=False,
                   enable_asserts=True, num_devices=NCORE)

    # x packed 128-partition: row p = batch b + 64*(c//16), col cc = c%16
    x_in = nc.dram_tensor("x_sh", [128, HC, N], BF16, kind="ExternalInput")
    wt_in = nc.dram_tensor("wt", [CL, 128, 4, N], BF16, kind="ExternalInput")
    ct_in = nc.dram_tensor("ct", [NL, 128, 4, C], BF16, kind="ExternalInput")
    b1t_in = nc.dram_tensor("b1t", [128, 2, CL], F32, kind="ExternalInput")
    b2t_in = nc.dram_tensor("b2t", [128, 2, CL], F32, kind="ExternalInput")
    bc1t_in = nc.dram_tensor("bc1t", [128, 2, NL], F32, kind="ExternalInput")
    bc2t_in = nc.dram_tensor("bc2t", [128, 2, NL], F32, kind="ExternalInput")
    ones_in = nc.dram_tensor("onesel", [128, HC, HC], BF16, kind="ExternalInput")
    id64_in = nc.dram_tensor("id64", [128, 64], BF16, kind="ExternalInput")

    ybuf = nc.dram_tensor("ybuf", [C, NL, B], BF16, kind="ExternalOutput")
    dbg = False

    with tile.TileContext(nc) as tc:
        with tc.tile_pool(name="const", bufs=1) as const, \
             tc.tile_pool(name="wpool", bufs=4) as wpool, \
             tc.tile_pool(name="act", bufs=6) as act, \
             tc.tile_pool(name="small", bufs=2) as small, \
             tc.tile_pool(name="dram", bufs=1, space="DRAM") as dram, \
             tc.tile_pool(name="ps", bufs=2, space="PSUM") as ps, \
             tc.tile_pool(name="pstat", bufs=1, space="PSUM") as pstat:

            # exchange buffers: one 16-channel group, then two 8-channel
            QBLK = 8 * CSTR
            GSIZE = (BLK, QBLK, QBLK)
            send = [dram.tile([NCORE, GSIZE[i]], BF16, name=f"send{i}",
                              tag=f"send{i}") for i in range(3)]
            recv = [dram.tile([NCORE, GSIZE[i]], BF16, name=f"recv{i}",
                              tag=f"recv{i}") for i in range(3)]
            scr1 = [dram.tile([HC * 2 * B], BF16, name=f"scr1{h}",
                              tag=f"scr1{h}") for h in range(2)]
            scr2 = [dram.tile([HC * 2 * B], BF16, name=f"scr2{i}",
                              tag=f"scr2{i}") for i in range(3)]

            # ---- constants / persistent tiles ----
            x_a = const.tile([128, HC, N], BF16)
            nc.sync.dma_start(out=x_a[:], in_=x_in[:])
            id64 = const.tile([128, 64], BF16)
            nc.scalar.dma_start(out=id64[:], in_=id64_in[:])
            onesel = const.tile([128, HC, HC], BF16)
            nc.scalar.dma_start(out=onesel[:], in_=ones_in[:])
            b1t = const.tile([128, 2, CL], F32)
            nc.scalar.dma_start(out=b1t[:], in_=b1t_in[:])
            b2t = const.tile([128, 2, CL], F32)
            nc.scalar.dma_start(out=b2t[:], in_=b2t_in[:])
            bc1t = const.tile([128, 2, NL], F32)
            nc.scalar.dma_start(out=bc1t[:], in_=bc1t_in[:])
            bc2t = const.tile([128, 2, NL], F32)
            nc.scalar.dma_start(out=bc2t[:], in_=bc2t_in[:])
            eps64 = const.tile([64, 1], F32)
            nc.vector.memset(eps64[:], EPS)

            # combined [c, kb, (sq|val), b]: t=0 squares, t=1 values
            xt_all = const.tile([128, CL, 2, 2, 64], BF16)
            u_bf = const.tile([128, CL, 2, 2, 64], BF16)
            rn = [const.tile([128, HC, 2, 64], BF16, name=f"rn{h}",
                             tag=f"rn{h}") for h in range(2)]
            yn_all = const.tile([128, 2, NL, 64], BF16)
            ub = [const.tile([128, NLR, 64], BF16, name=f"ub{h}",
                             tag=f"ub{h}") for h in range(2)]
            y_stage = const.tile([128, 2, NL, 64], BF16)

            st1 = pstat.tile([HC, 2, 64], F32, tag="st1")  # LN1 half A
            st2 = pstat.tile([HC, 2, 64], F32, tag="st2")  # LN1 half B
            stat1 = [st1, st2]

            # ---- xT pass: transpose + LN1 stat accumulation ----
            # one accumulation group per PSUM bank: moving = [x^2 | x]
            def ln1_stats(j):
                stt = stat1[j // HC]
                cl = j % HC
                for kb in range(2):
                    nc.tensor.matmul(
                        stt[:, :, :].rearrange("p a b -> p (a b)"),
                        onesel[:, cl, :],
                        xt_all[:, j, kb, :, :].rearrange("p a b -> p (a b)"),
                        start=(cl == 0 and kb == 0),
                        stop=(cl == HC - 1 and kb == 1),
                        skip_group_check=True)

            for c in range(CL):
                xtp = ps.tile([128, 2, 64], F32, tag="xtp")
                p0 = 64 * (c // HC)
                for blk in range(2):
                    nc.tensor.matmul(
                        xtp[:, blk, :],
                        x_a[p0:p0 + 64, c % HC, blk * 128:(blk + 1) * 128],
                        id64[p0:p0 + 64, :], start=True, stop=True)
                nc.vector.tensor_copy(out=xt_all[:, c, :, 1, :], in_=xtp[:])
                nc.vector.tensor_mul(out=xt_all[:, c, :, 0, :],
                                     in0=xt_all[:, c, :, 1, :],
                                     in1=xt_all[:, c, :, 1, :])
                if c > 1:
                    ln1_stats(c - 2)
            ln1_stats(CL - 2)
            ln1_stats(CL - 1)

            # ---- LN1 finalize per half: rstd/nmr -> replicated rn tiles ----
            def ln_finalize(stt, sbf):
                """sbf[:,0,:] = -mu*rstd (nmr), sbf[:,1,:] = rstd (bf16)."""
                mu = small.tile([HC, 64], F32, tag="mu")
                nc.vector.tensor_scalar(out=mu[:], in0=stt[:, 1, :],
                                        scalar1=1.0 / N, scalar2=None, op0=MUL)
                esq = small.tile([HC, 64], F32, tag="esq")
                nc.vector.tensor_scalar(out=esq[:], in0=stt[:, 0, :],
                                        scalar1=1.0 / N, scalar2=None, op0=MUL)
                var = small.tile([HC, 64], F32, tag="var")
                nc.vector.tensor_mul(out=var[:], in0=mu[:], in1=mu[:])
                nc.vector.tensor_sub(out=var[:], in0=esq[:], in1=var[:])
                rstd = small.tile([HC, 64], F32, tag="rstd")
                nc.scalar.activation(out=rstd[:], in_=var[:], func=SQRT,
                                     bias=eps64[0:HC, :], scale=1.0)
                with nc.allow_low_precision(reason="stats used in bf16"):
                    nc.vector.reciprocal(out=sbf[:, 1, :], in_=rstd[:])
                nc.vector.scalar_tensor_tensor(
                    out=sbf[:, 0, :], in0=mu[:], scalar=-1.0,
                    in1=sbf[:, 1, :], op0=MUL, op1=MUL)

            for h in range(2):
                s1bf = small.tile([HC, 2, 64], BF16, tag="s1bf")
                ln_finalize(stat1[h], s1bf)
                nc.gpsimd.dma_start(
                    out=bass.AP(tensor=scr1[h].tensor, offset=0,
                                ap=[[1, HC * 2 * B]]),
                    in_=s1bf[:])
                # replicate [16c,2t,64b] stats across all 128 partitions
                nc.gpsimd.dma_start(
                    out=rn[h][:],
                    in_=bass.AP(tensor=scr1[h].tensor, offset=0,
                                ap=[[0, 128], [1, HC * 2 * B]]))

            # ---- main token-mix loop, software-pipelined ----
            # rn layout per partition: [cl, t, b] with t=0 nmr, t=1 rstd
            def emit_xn(c):
                h, cl = c // HC, c % HC
                z = act.tile([128, 2, 64], BF16, tag="z")
                nc.vector.tensor_mul(
                    out=z[:], in0=xt_all[:, c, :, 1, :],
                    in1=rn[h][:, cl, 1:2, :].broadcast_to([128, 2, 64]))
                nc.vector.tensor_add(
                    out=z[:], in0=z[:],
                    in1=rn[h][:, cl, 0:1, :].broadcast_to([128, 2, 64]))
                return z

            w_t, z_t, hs_t = {}, {}, {}

            def emit_w(c):
                w12 = wpool.tile([128, 4, N], BF16, tag="w")
                nc.sync.dma_start(out=w12[:], in_=wt_in[c])
                w_t[c] = w12

            def emit_fc1(c):
                hpre = ps.tile([128, 2, 64], F32, tag="hpre")
                for mb in range(2):
                    for nb in range(2):
                        nc.tensor.matmul(
                            hpre[:, mb, :],
                            w_t[c][:, nb, mb * 128:(mb + 1) * 128],
                            z_t[c][:, nb, :], start=(nb == 0), stop=(nb == 1))
                hs = act.tile([128, 2, 64], BF16, tag="h")
                for mb in range(2):
                    nc.scalar.activation(out=hs[:, mb, :], in_=hpre[:, mb, :],
                                         func=gelu_func,
                                         bias=b1t[:, mb, c:c + 1])
                hs_t[c] = hs

            def emit_fc2(c):
                tokp = ps.tile([128, 2, 64], F32, tag="tokp")
                for kb in range(2):
                    for mb in range(2):
                        nc.tensor.matmul(
                            tokp[:, kb, :],
                            w_t[c][:, 2 + mb, kb * 128:(kb + 1) * 128],
                            hs_t[c][:, mb, :], start=(mb == 0), stop=(mb == 1))
                del w_t[c], hs_t[c]
                if skip_b2:
                    nc.vector.tensor_add(out=u_bf[:, c, :, 1, :],
                                         in0=xt_all[:, c, :, 1, :],
                                         in1=tokp[:])
                else:
                    t = act.tile([128, 2, 64], F32, tag="t")
                    for kb in range(2):
                        nc.vector.tensor_scalar(
                            out=t[:, kb, :], in0=tokp[:, kb, :],
                            scalar1=b2t[:, kb, c:c + 1], scalar2=None, op0=ADD)
                    nc.vector.tensor_add(out=u_bf[:, c, :, 1, :],
                                         in0=xt_all[:, c, :, 1, :], in1=t[:])
                nc.vector.tensor_mul(out=u_bf[:, c, :, 0, :],
                                     in0=u_bf[:, c, :, 1, :],
                                     in1=u_bf[:, c, :, 1, :])

            def emit_ln2(c):
                stt = stat1[c // HC]   # st1/st2 slots reused for LN2
                cl = c % HC
                for kb in range(2):
                    nc.tensor.matmul(
                        stt[:, :, :].rearrange("p a b -> p (a b)"),
                        onesel[:, cl, :],
                        u_bf[:, c, kb, :, :].rearrange("p a b -> p (a b)"),
                        start=(cl == 0 and kb == 0),
                        stop=(cl == HC - 1 and kb == 1),
                        skip_group_check=True)

            def emit_ship(h):
                """LN2 finalize + stage u + stats, then AllToAll + restage."""
                sl = slice(h * HC, (h + 1) * HC)
                s2bf = small.tile([HC, 2, 64], BF16, tag="s2bf")
                ln_finalize(stat1[h], s2bf)
                nc.sync.dma_start(
                    out=bass.AP(tensor=scr2[h].tensor, offset=0,
                                ap=[[1, HC * 2 * B]]),
                    in_=s2bf[:])
                nc.scalar.dma_start(
                    out=bass.AP(tensor=send[h].tensor, offset=STOFF,
                                ap=[[BLK, NCORE], [CSTR, HC], [B, 2], [1, B]]),
                    in_=bass.AP(tensor=scr2[h].tensor, offset=0,
                                ap=[[0, NCORE], [1, HC * 2 * B]]))
                rings = [nc.sync, nc.scalar]
                for kb in range(2):
                    for jr in range(4):
                        rings[kb].dma_start(
                            out=bass.AP(tensor=send[h].tensor,
                                        offset=(kb * 4 + jr) * BLK,
                                        ap=[[B, NL], [CSTR, HC], [1, B]]),
                            in_=u_bf[jr * 32:(jr + 1) * 32, sl, kb, 1, :])
                nc.gpsimd.collective_compute(
                    "AllToAll", mybir.AluOpType.bypass,
                    replica_groups=[list(range(NCORE))],
                    ins=[send[h].opt()], outs=[recv[h].opt()])
                nc.gpsimd.dma_start(
                    out=ub[h][:],
                    in_=bass.AP(tensor=recv[h].tensor, offset=0,
                                ap=[[CSTR, 128], [B, NLR], [1, B]]))

            def emit_yn(gi):
                """yn = u*rstd2 + nmr2 for group gi's 64 ub rows."""
                h, p0 = gi // 2, 64 * (gi % 2)
                pe = p0 + 64
                nc.vector.tensor_mul(
                    out=yn_all[p0:pe, h, :, :], in0=ub[h][p0:pe, 0:NL, :],
                    in1=ub[h][p0:pe, NL + 1:NL + 2, :].broadcast_to(
                        [64, NL, 64]))
                nc.vector.tensor_add(
                    out=yn_all[p0:pe, h, :, :], in0=yn_all[p0:pe, h, :, :],
                    in1=ub[h][p0:pe, NL:NL + 1, :].broadcast_to(
                        [64, NL, 64]))

            for i in range(5):
                emit_w(i)
            z_t[0] = emit_xn(0)
            for c in range(CL):
                if c + 5 < CL:
                    emit_w(c + 5)
                if c + 1 < CL:
                    z_t[c + 1] = emit_xn(c + 1)
                emit_fc1(c)
                del z_t[c]
                if c == 2:
                    ln1_finalize(1)   # rn_b needed from xn(16) only
                if c >= 1:
                    emit_fc2(c - 1)
                if c >= 2:
                    emit_ln2(c - 2)
                if c == 10:
                    emit_fin0(0)
                if c == 17:
                    emit_ship(0)
                if c in (20, 23, 26, 29):
                    g = (c - 20) // 3
                    emit_yn(0, g * 8, (g + 1) * 8)
            emit_fc2(CL - 1)
            emit_ln2(CL - 2)
            emit_ln2(CL - 1)
            emit_ship(1)

            # ---- phase 2: channel mixing per patch, software-pipelined ----
            c_t, h2s_t, chp_t = {}, {}, {}

            def emit_ct(nl):
                c12 = wpool.tile([128, 4, C], BF16, tag="w")
                nc.sync.dma_start(out=c12[:], in_=ct_in[nl])
                c_t[nl] = c12

            def emit_cfc1(nl):
                h2p = ps.tile([128, 2, 64], F32, tag=("hpre", "xtp")[nl % 2])
                for ob in range(2):
                    for cb in range(2):
                        nc.tensor.matmul(
                            h2p[:, ob, :],
                            c_t[nl][:, cb, ob * 128:(ob + 1) * 128],
                            yn_all[:, cb, nl, :],
                            start=(cb == 0), stop=(cb == 1))
                h2s = act.tile([128, 2, 64], BF16, tag="h")
                for ob in range(2):
                    nc.scalar.activation(out=h2s[:, ob, :], in_=h2p[:, ob, :],
                                         func=gelu_func,
                                         bias=bc1t[:, ob, nl:nl + 1])
                h2s_t[nl] = h2s

            def emit_cfc2(nl):
                if nl % 2 == 0:
                    chp = ps.tile([128, 2, 64], F32, tag="tokp")
                else:
                    chp = pstat.tile([128, 2, 64], F32,
                                     tag=("st1", "st2")[(nl // 2) % 2])
                for hb in range(2):
                    for ob in range(2):
                        nc.tensor.matmul(
                            chp[:, hb, :],
                            c_t[nl][:, 2 + ob, hb * 128:(hb + 1) * 128],
                            h2s_t[nl][:, ob, :],
                            start=(ob == 0), stop=(ob == 1))
                del c_t[nl], h2s_t[nl]
                for hb in range(2):
                    if skip_bc2:
                        nc.vector.tensor_add(out=y_stage[:, hb, nl, :],
                                             in0=chp[:, hb, :],
                                             in1=ub[hb][:, nl, :])
                    else:
                        t3 = act.tile([128, 64], F32, tag="t3")
                        nc.vector.tensor_scalar(
                            out=t3[:], in0=chp[:, hb, :],
                            scalar1=bc2t[:, hb, nl:nl + 1], scalar2=None,
                            op0=ADD)
                        nc.vector.tensor_add(out=y_stage[:, hb, nl, :],
                                             in0=t3[:], in1=ub[hb][:, nl, :])

            def emit_out(g):
                # ybuf is [C, NL, B]; row p=16j+cl of half hb -> channel
                # 32j+16hb+cl at address (32j+16hb+cl)*NL*B
                for hb in range(2):
                    nc.scalar.dma_start(
                        out=bass.AP(tensor=ybuf,
                                    offset=hb * HC * NL * B + g * HC * B,
                                    ap=[[CL * NL * B, NCORE],
                                        [NL * B, HC], [1, HC * B]]),
                        in_=y_stage[:, hb, g * HC:(g + 1) * HC, :])

            emit_ct(0)
            emit_ct(1)
            emit_ct(2)
            emit_yn(1, 0, NL)
            for nl in range(NL):
                if nl + 3 < NL:
                    emit_ct(nl + 3)
                emit_cfc1(nl)
                if nl >= 1:
                    emit_cfc2(nl - 1)
                if nl == 17:
                    emit_out(0)
            emit_cfc2(NL - 1)
            emit_out(1)

    nc.finalize()
    return nc


def prep_inputs(x, g1, be1, g2, be2, tw1, tb1, tw2, tb2, cw1, cb1, cw2, cb2):
    """Host-side sharding + weight folding. Returns in_maps for the 8 cores."""
    f = np.float32
    bf = ml_dtypes.bfloat16
    x = np.asarray(x, f)
    g1, be1, g2, be2 = (np.asarray(a, f) for a in (g1, be1, g2, be2))
    tw1, tb1, tw2, tb2 = (np.asarray(a, f) for a in (tw1, tb1, tw2, tb2))
    cw1, cb1, cw2, cb2 = (np.asarray(a, f) for a in (cw1, cb1, cw2, cb2))

    # token-mix fc1: fold g1 into weights, be1 into bias; lhsT layout [c, n, m]
    w1t = (tw1 * g1[None, None, :]).transpose(0, 2, 1)            # [C, N, N]
    bias1 = tb1 + np.einsum('n,cmn->cm', be1, tw1)                # [C, M]
    w2t = tw2.transpose(0, 2, 1)                                  # [c, m, k]
    t1r = w1t.reshape(C, 2, 128, N)
    t2r = w2t.reshape(C, 2, 128, N)
    wt = np.ascontiguousarray(
        np.stack([t1r[:, 0], t1r[:, 1], t2r[:, 0], t2r[:, 1]],
                 axis=2)).astype(bf)                              # [C, 128, 4, N]

    # recv channel order: row (hb, q, j, cl) -> channel 32j+16hb+8q+cl
    perm = np.array([32 * j + 16 * hb + 8 * q + cl
                     for hb in range(2) for q in range(2)
                     for j in range(NCORE) for cl in range(8)])    # [256]

    # channel-mix fc1: fold g2 (per-patch scalar) into cw1, be2 into bias
    c1t = (cw1 * g2[:, None, None]).transpose(0, 2, 1)            # [N, C_in, O]
    c1t = c1t[:, perm, :]                                         # permute c_in
    biasc1 = cb1 + be2[:, None] * cw1.sum(axis=2)                 # [N, O]
    c2t = cw2.transpose(0, 2, 1)[:, :, perm]                      # [n, o, k_perm]
    c1r = c1t.reshape(N, 2, 128, C)
    c2r = c2t.reshape(N, 2, 128, C)
    ct = np.ascontiguousarray(
        np.stack([c1r[:, 0], c1r[:, 1], c2r[:, 0], c2r[:, 1]],
                 axis=2)).astype(bf)                              # [N, 128, 4, C]
    bc2p = cb2[:, perm]                                           # [N, K]

    idx = np.arange(64)
    onesel = np.zeros((128, HC, HC), bf)
    onesel[:, idx[:HC], idx[:HC]] = 1.0
    id64 = np.tile(np.eye(64, dtype=bf), (2, 1))

    def fold_bias(bm):   # [G, 256] -> [128, 2, G]
        return np.ascontiguousarray(bm.T.reshape(2, 128, -1).transpose(1, 0, 2))

    in_maps = []
    for m in range(NCORE):
        cs = slice(m * CL, (m + 1) * CL)
        ns = slice(m * NL, (m + 1) * NL)
        xc = x[:, cs, :]                                  # [B, CL, N]
        xpk = np.concatenate([xc[:, 0:HC, :], xc[:, HC:CL, :]],
                             axis=0)                      # [128, HC, N]
        in_maps.append({
            "x_sh": np.ascontiguousarray(xpk).astype(bf),
            "wt": np.ascontiguousarray(wt[cs]),
            "ct": np.ascontiguousarray(ct[ns]),
            "b1t": fold_bias(bias1[cs]),
            "b2t": fold_bias(tb2[cs]),
            "bc1t": fold_bias(biasc1[ns]),
            "bc2t": fold_bias(bc2p[ns]),
            "onesel": onesel,
            "id64": id64,
        })
    return in_maps


def assemble_output(results):
    """results: list of per-core dicts with 'ybuf' [C, NL, B] -> y [B, C, N]."""
    y = np.empty((B, C, N), np.float32)
    for k in range(NCORE):
        y[:, :, k * NL:(k + 1) * NL] = \
            results[k]["ybuf"].astype(np.float32).transpose(2, 0, 1)
    return y


_PROGRAMS = {}


def get_program(skip_b2, skip_bc2):
    key = (skip_b2, skip_bc2)
    if key not in _PROGRAMS:
        _PROGRAMS[key] = build_program(skip_b2=skip_b2, skip_bc2=skip_bc2)
    return _PROGRAMS[key]


def kernel(**inputs):
    skip_b2 = not np.any(np.asarray(inputs["tb2"]))
    skip_bc2 = not np.any(np.asarray(inputs["cb2"]))
    prog = get_program(skip_b2, skip_bc2)
    in_maps = prep_inputs(**inputs)
    res = run_bass_kernel_spmd(prog, in_maps, list(range(NCORE)))
    return assemble_output(res.results)


if __name__ == "__main__":
    from scipy.special import erf

    rng = np.random.RandomState(0)
    s = 0.02
    inputs = dict(
        x=rng.randn(B, C, N).astype(np.float32),
        g1=np.ones(N, np.float32), be1=np.zeros(N, np.float32),
        g2=np.ones(N, np.float32), be2=np.zeros(N, np.float32),
        tw1=(rng.randn(C, N, N) * s).astype(np.float32),
        tb1=np.zeros((C, N), np.float32),
        tw2=(rng.randn(C, N, N) * s).astype(np.float32),
        tb2=np.zeros((C, N), np.float32),
        cw1=(rng.randn(N, C, C) * s).astype(np.float32),
        cb1=np.zeros((N, C), np.float32),
        cw2=(rng.randn(N, C, C) * s).astype(np.float32),
        cb2=np.zeros((N, C), np.float32),
    )

    def np_ref(x, g1, be1, g2, be2, tw1, tb1, tw2, tb2, cw1, cb1, cw2, cb2):
        def ln(z, g, b):
            mu = z.mean(-1, keepdims=True)
            var = z.var(-1, keepdims=True)
            return (z - mu) / np.sqrt(var + EPS) * g + b
        def gelu(v):
            return v * 0.5 * (1 + erf(v / np.sqrt(2.0)))
        xn = ln(x, g1, be1)
        h = gelu(np.einsum('bcn,cmn->bcm', xn, tw1) + tb1[None])
        tok = np.einsum('bcm,ckm->bck', h, tw2) + tb2[None]
        x = x + tok
        yn = ln(x, g2, be2)
        h2 = gelu(np.einsum('bcn,noc->bon', yn, cw1) + cb1.T[None])
        ch = np.einsum('bon,nko->bkn', h2, cw2) + cb2.T[None]
        return x + ch

    exp = np_ref(**{k: v.astype(np.float64) for k, v in inputs.items()})
    got = kernel(**inputs)
    err = np.abs(got - exp)
    rel = err.max() / np.abs(exp).max()
    print(f"abs err: {err.max():.3e}  rel(absmax): {rel:.3e}")
